# revision 25
# baseline (speedup 1.0000x reference)
"""Batch semi-hard triplet loss (cosine distance) on 8 Trainium2 NeuronCores.

Strategy (data-parallel over rows, per sharding hint):
  - Host: sort rows by label; core c takes sorted rows [1024c, 1024(c+1)) in
    8 exact 128-row M-tiles; columns rotated per core so its rows' class
    columns sit in the first PSUM group of each M-tile.
  - Device (per core, uniform SPMD program):
      * normalize embeddings (squares, one-hot column-sum matmuls, sqrt,
        reciprocal, one-hot broadcast matmuls, column scale);
      * per M-tile m: 16 matmuls (4-bank PSUM groups). Class-column poison
        (-2) is applied ON THE PE via small rank-per-class accumulate
        matmuls (lhsT = -2*row-indicators, rhs = col-indicators), so the
        diag group needs no mask adds. t_p (min positive-class dot) comes
        from a narrow poisoned window min on the first group.
        Then three engines split the threshold-max reduction
        q = max{dot < t_p}:
          - Act banks: u = 1/(dot - t_p) (Reciprocal w/ per-partition
            bias), bf16; float min-tree over u -> r1 (min u).
          - Pool banks: y = (dot min t_p) - t_p (one fused gpsimd
            tensor_scalar), bf16: candidates are negative, others +0;
            signed-int16 bit-pattern min over y picks the largest dot
            strictly below t_p (sign bit wraps the threshold).
          - DVE: runs both min-trees (u float-min, y int16-bits-min),
            one M-tile behind.
  - Host: q = max(t_p + 1/r1, t_p + y); per-row loss epilogue in f64;
    rows with no semi-hard candidate in the margin window (or near the
    branch boundary) are recomputed exactly in f32 numpy; mean over valid.
"""

import numpy as np
import ml_dtypes

B = 8192
D = 128
MARGIN = 0.2
NCORES = 8
NT = 512            # N-tile width (one PSUM bank of fp32)
N_NT = B // NT      # 16
MT = 128            # M-tile rows
NMT = B // NCORES // MT  # 8 m-tiles per core
GW = 2              # N-tiles per PSUM group tile (4-deep rotation)
POIS = -2.0         # class-column poison (exactly representable in bf16)

# bank families: Act does recip on banks 2..11; Pool shifts banks 0,1,12..15
ACT_BANKS = list(range(2, 12))
POOL_BANKS = [0, 1, 12, 13, 14, 15]
NA = len(ACT_BANKS)
NP_ = len(POOL_BANKS)

BF16 = ml_dtypes.bfloat16

_CACHE = {}


# --------------------------------------------------------------------------
# host-side planning (pure layout, computed from labels)
# --------------------------------------------------------------------------
def _plan(labels: np.ndarray):
    order = np.argsort(labels, kind="stable")
    slab = labels[order]
    bounds = np.flatnonzero(np.r_[True, slab[1:] != slab[:-1], True])
    cls_start, cls_end = bounds[:-1], bounds[1:]
    row_s = np.empty(B, dtype=np.int64)
    row_e = np.empty(B, dtype=np.int64)
    for s, e in zip(cls_start, cls_end):
        row_s[s:e] = s
        row_e[s:e] = e

    rows_per_core = B // NCORES
    cores = []
    for c in range(NCORES):
        r0 = c * rows_per_core
        base = int(row_s[r0])  # start of first class -> no wraparound
        diag = []
        for m in range(NMT):
            rr = slice(r0 + m * MT, r0 + (m + 1) * MT)
            s = row_s[rr] - base
            e = row_e[rr] - base
            dts = sorted(set((s // NT).tolist()) | set(((e - 1) // NT).tolist()))
            diag.append(dts)
        cores.append(dict(r0=r0, base=base, diag=diag))
    # unify diag sets across cores so all 8 run one compiled program
    uni = [
        sorted(set().union(*[set(pc["diag"][m]) for pc in cores]))
        for m in range(NMT)
    ]
    for pc in cores:
        pc["diag"] = uni
    # per (m, diag tile): narrow column window [c0, c1) within the bank that
    # contains every class column of the tile's rows, across all cores
    wins = []
    for m in range(NMT):
        wm_ = []
        for d in uni[m]:
            c0, c1 = NT, 0
            for pc in cores:
                rr = slice(pc["r0"] + m * MT, pc["r0"] + (m + 1) * MT)
                s = np.maximum(row_s[rr] - pc["base"] - d * NT, 0)
                e = np.minimum(row_e[rr] - pc["base"] - d * NT, NT)
                ok = s < e
                if ok.any():
                    c0 = min(c0, int(s[ok].min()))
                    c1 = max(c1, int(e[ok].max()))
            if c1 <= c0:
                c0, c1 = 0, NT
            wm_.append((c0, c1))
        wins.append(wm_)
    # max classes per (m, diag-tile) block across cores (pois matmul k-dim)
    cp = 1
    for c in range(NCORES):
        pc = cores[c]
        r0, base = pc["r0"], pc["base"]
        for m in range(NMT):
            rr = slice(r0 + m * MT, r0 + (m + 1) * MT)
            ss = row_s[rr]
            for d in uni[m]:
                lo, hi = base + d * NT, base + (d + 1) * NT
                # classes whose column range intersects the bank
                cls = set()
                for g in range(rr.start, rr.stop):
                    if row_s[g] < hi and row_e[g] > lo:
                        cls.add(int(row_s[g]))
                cp = max(cp, len(cls))
    return dict(
        order=order, row_s=row_s, row_e=row_e, cores=cores, diag=uni,
        wins=wins, cp=cp,
    )


def _build_core_inputs(emb_sorted: np.ndarray, plan, c: int):
    """Returns (xt_rot [D,B], xb [D,1024], pl [CP, nblk*MT],
    pr [CP, nblk*NT]) all bf16."""
    pc = plan["cores"][c]
    base, r0 = pc["base"], pc["r0"]
    rows_per_core = B // NCORES
    row_s, row_e = plan["row_s"], plan["row_e"]
    cp = plan["cp"]

    rot = np.r_[np.arange(base, B), np.arange(0, base)]
    xt_rot = np.ascontiguousarray(emb_sorted[rot].T).astype(BF16)
    xb = np.ascontiguousarray(emb_sorted[r0 : r0 + rows_per_core].T).astype(BF16)

    # poison matmul blocks: per (m, d in diag[m]):
    #   pl[k, i] = -2 if m-tile row i in class k else 0     [CP, MT]
    #   pr[k, j] = 1 if bank-d col j in class k else 0      [CP, NT]
    nblk = sum(len(d) for d in pc["diag"])
    pl = np.zeros((cp, nblk * MT), np.float32)
    pr = np.zeros((cp, nblk * NT), np.float32)
    bi = 0
    for m in range(NMT):
        for d in pc["diag"][m]:
            lo, hi = base + d * NT, base + (d + 1) * NT
            cls = {}
            for r in range(MT):
                g = r0 + m * MT + r
                s, e = int(row_s[g]), int(row_e[g])
                if s < hi and e > lo:
                    k = cls.setdefault(s, len(cls))
                    pl[k, bi * MT + r] = POIS
                    cs, ce = max(s - lo, 0), min(e - lo, NT)
                    pr[k, bi * NT + cs : bi * NT + ce] = 1.0
            assert len(cls) <= cp
            bi += 1
    pl = pl.astype(BF16)
    pr = pr.astype(BF16)

    # one-hot routing blocks for the n2 colsum matmuls: batch A (10 tiles)
    # then batch B (8 tiles); block k is [D, NR] with ones in column k
    NRA, NRB = 10, 8
    oh = np.zeros((D, NRA * NRA + NRB * NRB), np.float32)
    for k in range(NRA):
        oh[:, k * NRA + k] = 1.0
    for k in range(NRB):
        oh[:, NRA * NRA + k * NRB + k] = 1.0
    oh = oh.astype(BF16)
    return xt_rot, xb, pl, pr, oh


# --------------------------------------------------------------------------
# device program
# --------------------------------------------------------------------------
def _raw_recip_bias(nc, out, in_, bias_ap):
    import concourse.mybir as mybir

    eng = nc.scalar
    ins = [
        eng.lower_ap(in_),
        eng.lower_ap(bias_ap),
        mybir.ImmediateValue(dtype=mybir.dt.float32, value=1.0),  # scale
        mybir.ImmediateValue(dtype=mybir.dt.float32, value=0.0),  # alpha
    ]
    return eng.add_instruction(
        mybir.InstActivation(
            name=f"I-{nc.next_id()}",
            func=mybir.ActivationFunctionType.Reciprocal,
            ins=ins,
            outs=[eng.lower_ap(out)],
        )
    )


def _build_bass(diag, wins, cp):
    import concourse.bacc as bacc
    import concourse.mybir as mybir
    from concourse.tile import TileContext

    f32 = mybir.dt.float32
    bf16 = mybir.dt.bfloat16
    i16 = mybir.dt.int16
    Alu = mybir.AluOpType
    Act = mybir.ActivationFunctionType
    NOH = N_NT + 2
    NBC = NMT * MT  # xb columns (1024)
    nblk = sum(len(d) for d in diag)

    nc = bacc.Bacc("TRN2", target_bir_lowering=False, debug=False, num_devices=NCORES)

    xt_d = nc.dram_tensor("xt", [D, B], bf16, kind="ExternalInput").ap()
    xb_d = nc.dram_tensor("xb", [D, NBC], bf16, kind="ExternalInput").ap()
    pl_d = nc.dram_tensor("pl", [cp, nblk * MT], bf16, kind="ExternalInput").ap()
    pr_d = nc.dram_tensor("pr", [cp, nblk * NT], bf16, kind="ExternalInput").ap()
    NRA, NRB = 10, 8
    oh_d = nc.dram_tensor(
        "oh", [D, NRA * NRA + NRB * NRB], bf16, kind="ExternalInput"
    ).ap()
    out_d = nc.dram_tensor("out", [MT, 2 * NMT], f32, kind="ExternalOutput").ap()
    outy_d = nc.dram_tensor("outy", [MT, NMT], bf16, kind="ExternalOutput").ap()

    # diag-block flat index per (m, d)
    blkof = {}
    bi = 0
    for m in range(NMT):
        for j, d in enumerate(diag[m]):
            blkof[(m, d)] = bi
            bi += 1

    with TileContext(nc) as tc:
        with (
            tc.tile_pool(name="big", bufs=1) as big,
            tc.tile_pool(name="upool", bufs=2) as upool,
            tc.tile_pool(name="ypool", bufs=2) as ypool,
            tc.tile_pool(name="scr", bufs=2) as scr,
            tc.tile_pool(name="sm", bufs=6) as smp,
            tc.tile_pool(name="psw", bufs=8 // GW, space="PSUM") as psw,
        ):
            # ---------------- setup: load + normalize (pipelined) -----------
            oh = big.tile([D, NRA * NRA + NRB * NRB], bf16, tag="oh")
            nc.sync.dma_start(oh[:], oh_d)
            xb = big.tile([D, NBC], bf16, tag="xb")
            nc.sync.dma_start(xb[:], xb_d)
            pl = big.tile([cp, nblk * MT], bf16, tag="pl")
            nc.sync.dma_start(pl[:], pl_d)
            pr = big.tile([cp, nblk * NT], bf16, tag="pr")
            nc.sync.dma_start(pr[:], pr_d)
            xt = big.tile([D, B], bf16, tag="xt")
            for j in range(8):
                sl = slice(j * (B // 8), (j + 1) * (B // 8))
                nc.sync.dma_start(xt[:, sl], xt_d[:, sl])

            sq = big.tile([D, NOH * NT], bf16, tag="sq")
            xtn = big.tile([D, B], bf16, tag="xtn")
            xbn = big.tile([D, NBC], bf16, tag="xbn")
            rn128 = big.tile([MT, NOH * NT], bf16, tag="rn128")
            outb = big.tile([MT, 2 * NMT], f32, tag="outb")
            outy = big.tile([MT, NMT], bf16, tag="outy")
            ntpall = big.tile([MT, NMT], f32, tag="ntpall")

            # rn128 column layout: tiles [xb0, xb1, xt0..xt15] (NOH slots);
            # batch A = slots 0..9 (xb + xt tiles 0..7), batch B = 10..17.
            def norm_batch(slot0, oh0, chunks):
                """chunks: list of (slot, src_ap, ntiles, engine); squares in
                wide ops; one-hot n2 colsum matmuls accumulate into PSUM rows;
                batched rsqrt + bf16 copy; rearrange + doubling DMAs into
                rn128."""
                nrow = sum(ch[2] for ch in chunks)
                pn = psw.tile([MT, GW * NT], f32, tag="w", name="pn")
                k = 0
                for slot, src, ntiles, eng in chunks:
                    dst = sq[:, slot * NT : (slot + ntiles) * NT]
                    if eng == "act":
                        nc.scalar.activation(dst, src, Act.Square)
                    elif eng == "pool":
                        nc.gpsimd.tensor_tensor(dst, src, src, Alu.mult)
                    else:
                        nc.vector.tensor_tensor(dst, src, src, Alu.mult)
                    for t in range(ntiles):
                        nc.tensor.matmul(
                            pn[0:nrow, :NT],
                            oh[:, oh0 + k * nrow : oh0 + (k + 1) * nrow],
                            sq[:, (slot + t) * NT : (slot + t + 1) * NT],
                            start=(k == 0), stop=(k == nrow - 1),
                        )
                        k += 1
                s0 = smp.tile([NOH, NT], f32, tag="s0", name="s0", bufs=2)
                nc.scalar.activation(s0[0:nrow, :], pn[0:nrow, :NT], Act.Sqrt)
                r0 = smp.tile([NOH, NT], f32, tag="r0", name="r0", bufs=2)
                nc.vector.reciprocal(r0[0:nrow, :], s0[0:nrow, :])
                rb = smp.tile([NOH, NT], bf16, tag="rb", name="rb", bufs=2)
                nc.scalar.copy(rb[0:nrow, :], r0[0:nrow, :])
                c0, c1 = slot0 * NT, (slot0 + nrow) * NT
                for k in range(nrow):
                    nc.sync.dma_start(
                        rn128[0:1, c0 + k * NT : c0 + (k + 1) * NT],
                        rb[k : k + 1, :],
                    )
                p = 1
                while p < MT:
                    nc.sync.dma_start(
                        rn128[p : min(2 * p, MT), c0:c1], rn128[0:p, c0:c1]
                    )
                    p *= 2

            norm_batch(0, 0, [
                (0, xb[:], 2, "vector"),
                (2, xt[:, 0 : 4 * NT], 4, "act"),
                (6, xt[:, 4 * NT : 8 * NT], 4, "vector"),
            ])
            norm_batch(10, NRA * NRA, [
                (10, xt[:, 8 * NT : 12 * NT], 4, "act"),
                (14, xt[:, 12 * NT : 16 * NT], 4, "pool"),
            ])

            # scales: all-SBUF bf16 (DVE 2x mode)
            nc.vector.tensor_tensor(xbn[:], xb[:], rn128[:, 0 : 2 * NT], Alu.mult)
            for h in range(4):
                sl = slice(h * 4 * NT, (h + 1) * 4 * NT)
                nc.vector.tensor_tensor(
                    xtn[:, sl], xt[:, sl],
                    rn128[:, (2 + 4 * h) * NT : (6 + 4 * h) * NT], Alu.mult,
                )

            # ---------------- main loop over M-tiles ----------------
            # per-bank slot in the u (Act) / y (Pool) buffers
            uslot = {b: i for i, b in enumerate(ACT_BANKS)}
            yslot = {b: i for i, b in enumerate(POOL_BANKS)}

            pending_tree = None  # (u, y, m) of previous M-tile

            def emit_trees(u, y, m):
                # u float min-tree: [MT, NA*NT] bf16 -> r1 -> outb[:, NMT+m]
                w = NA * NT  # 5120
                t1 = scr.tile([MT, w // 2], bf16, tag="ut1")
                nc.vector.tensor_tensor(t1[:], u[:, : w // 2], u[:, w // 2 :], Alu.min)
                t2 = scr.tile([MT, w // 4], bf16, tag="ut2")
                nc.vector.tensor_tensor(
                    t2[:], t1[:, : w // 4], t1[:, w // 4 :], Alu.min
                )
                t3 = scr.tile([MT, w // 8], bf16, tag="ut3")
                nc.vector.tensor_tensor(
                    t3[:], t2[:, : w // 8], t2[:, w // 8 :], Alu.min
                )
                t4 = scr.tile([MT, w // 16], bf16, tag="ut4")
                nc.vector.tensor_tensor(
                    t4[:], t3[:, : w // 16], t3[:, w // 16 :], Alu.min
                )
                t5 = scr.tile([MT, w // 32], bf16, tag="ut5")
                nc.vector.tensor_tensor(
                    t5[:], t4[:, : w // 32], t4[:, w // 32 :], Alu.min
                )
                nc.vector.tensor_reduce(
                    outb[:, NMT + m : NMT + m + 1], t5[:],
                    axis=mybir.AxisListType.X, op=Alu.min,
                )
                # y int16-bits min-tree: [MT, NP_*NT] bf16 -> outy[:, m]
                wy = NP_ * NT  # 3072
                yi = y[:].bitcast(i16)
                s1 = scr.tile([MT, wy // 2], i16, tag="yt1")
                nc.vector.tensor_tensor(
                    s1[:], yi[:, : wy // 2], yi[:, wy // 2 :], Alu.min
                )
                s2 = scr.tile([MT, wy // 4], i16, tag="yt2")
                nc.vector.tensor_tensor(
                    s2[:], s1[:, : wy // 4], s1[:, wy // 4 :], Alu.min
                )
                s3 = scr.tile([MT, wy // 8], i16, tag="yt3")
                nc.vector.tensor_tensor(
                    s3[:], s2[:, : wy // 8], s2[:, wy // 8 :], Alu.min
                )
                s4 = scr.tile([MT, wy // 16], i16, tag="yt4")
                nc.vector.tensor_tensor(
                    s4[:], s3[:, : wy // 16], s3[:, wy // 16 :], Alu.min
                )
                nc.vector.tensor_reduce(
                    outy[:, m : m + 1].bitcast(i16), s4[:],
                    axis=mybir.AxisListType.X, op=Alu.min,
                )

            def emit_tp(m):
                """Narrow-window diag matmuls (+pois) into one PSUM tile,
                then the t_p chain: Pool window mins -> DVE finalize."""
                dts = diag[m]
                lhsT = xbn[:, m * MT : (m + 1) * MT]
                tpp = outb[:, m : m + 1]
                ntp = ntpall[:, m : m + 1]
                minis = {}
                for j, d in enumerate(dts):
                    if j % GW == 0:
                        mini = psw.tile([MT, GW * NT], f32, tag="w")
                        minis[j // GW] = mini
                    jj = j % GW
                    c0, c1 = wins[m][j]
                    w = c1 - c0
                    i = blkof[(m, d)]
                    nc.tensor.matmul(
                        mini[:, jj * NT : jj * NT + w],
                        lhsT, xtn[:, d * NT + c0 : d * NT + c1],
                        start=True, stop=False,
                    )
                    nc.tensor.matmul(
                        mini[:, jj * NT : jj * NT + w],
                        pl[:, i * MT : (i + 1) * MT],
                        pr[:, i * NT + c0 : i * NT + c1],
                        start=False, stop=True,
                    )
                ndts = len(dts)
                posm = smp.tile([MT, max(ndts, 1)], f32, tag="posm")
                for j, d in enumerate(dts):
                    c0, c1 = wins[m][j]
                    jj = j % GW
                    nc.vector.tensor_reduce(
                        posm[:, j : j + 1],
                        minis[j // GW][:, jj * NT : jj * NT + (c1 - c0)],
                        axis=mybir.AxisListType.X, op=Alu.min,
                    )
                if ndts == 1:
                    minpos = posm[:, 0:1]
                else:
                    mp = smp.tile([MT, 1], f32, tag="minpos")
                    nc.vector.tensor_reduce(
                        mp[:], posm[:], axis=mybir.AxisListType.X, op=Alu.min
                    )
                    minpos = mp[:]
                # t_p = min(minpos - POIS, 1)
                nc.vector.tensor_scalar(
                    tpp, minpos, -POIS, 1.0, Alu.add, Alu.min
                )
                nc.vector.tensor_scalar_mul(ntp, tpp, -1.0)

            emit_tp(0)

            for m in range(NMT):
                dts = diag[m]
                lhsT = xbn[:, m * MT : (m + 1) * MT]
                tpp = outb[:, m : m + 1]       # +t_p ptr (f32)
                ntp = ntpall[:, m : m + 1]     # -t_p ptr (f32)
                u = upool.tile([MT, NA * NT], bf16, tag="u")
                y = ypool.tile([MT, NP_ * NT], bf16, tag="y")

                for g in range(N_NT // GW):
                    wg = psw.tile([MT, GW * NT], f32, tag="w")
                    for k in range(GW):
                        t = GW * g + k
                        if t in dts:
                            nc.tensor.matmul(
                                wg[:, k * NT : (k + 1) * NT],
                                lhsT, xtn[:, t * NT : (t + 1) * NT],
                                start=True, stop=False,
                            )
                            i = blkof[(m, t)]
                            nc.tensor.matmul(
                                wg[:, k * NT : (k + 1) * NT],
                                pl[:, i * MT : (i + 1) * MT],
                                pr[:, i * NT : (i + 1) * NT],
                                start=False, stop=True,
                            )
                        else:
                            nc.tensor.matmul(
                                wg[:, k * NT : (k + 1) * NT],
                                lhsT, xtn[:, t * NT : (t + 1) * NT],
                            )

                    # consumers (t_p for this m was precomputed)
                    bank0 = GW * g
                    ab = [b for b in range(bank0, bank0 + GW) if b in uslot]
                    if ab:
                        k0, k1 = ab[0] - bank0, ab[-1] - bank0 + 1
                        _raw_recip_bias(
                            nc,
                            u[:, uslot[ab[0]] * NT : uslot[ab[-1]] * NT + NT],
                            wg[:, k0 * NT : k1 * NT],
                            ntp,
                        )
                    pb = [b for b in range(bank0, bank0 + GW) if b in yslot]
                    if pb:
                        k0, k1 = pb[0] - bank0, pb[-1] - bank0 + 1
                        nc.gpsimd.tensor_scalar(
                            y[:, yslot[pb[0]] * NT : yslot[pb[-1]] * NT + NT],
                            wg[:, k0 * NT : k1 * NT],
                            tpp, tpp, Alu.min, Alu.subtract,
                        )

                # next M-tile's t_p runs on PE/Pool/DVE while this one's
                # Act/Pool streams are still draining
                if m + 1 < NMT:
                    emit_tp(m + 1)
                if pending_tree is not None:
                    emit_trees(*pending_tree)
                pending_tree = (u, y, m)

            emit_trees(*pending_tree)

            nc.sync.dma_start(out_d, outb[:])
            nc.sync.dma_start(outy_d, outy[:])

    nc.compile()
    return nc


# --------------------------------------------------------------------------
# entry point
# --------------------------------------------------------------------------
def _prepare(embeddings, labels):
    emb = np.asarray(embeddings, dtype=np.float32)
    lab = np.asarray(labels).astype(np.int64)
    plan = _plan(lab)
    emb_sorted = emb[plan["order"]]
    cores = [_build_core_inputs(emb_sorted, plan, c) for c in range(NCORES)]
    return emb, lab, plan, cores


def _host_reduce(emb, lab, plan, outs):
    """outs: per core {"out": [128, 16] f32, "outy": [128, 8] bf16}."""
    order = plan["order"]
    slab = lab[order]
    rows_per_core = B // NCORES

    t_p = np.zeros(B, np.float64)
    r1 = np.zeros(B, np.float64)
    yw = np.zeros(B, np.float64)
    for c in range(NCORES):
        o = np.asarray(outs[c]["out"], np.float64)
        oy = np.asarray(outs[c]["outy"]).astype(np.float64)
        for m in range(NMT):
            rr = slice(c * rows_per_core + m * MT, c * rows_per_core + (m + 1) * MT)
            t_p[rr] = o[:, m]
            r1[rr] = o[:, NMT + m]
            yw[rr] = oy[:, m]

    with np.errstate(divide="ignore", invalid="ignore"):
        q1 = t_p + 1.0 / r1
    q2 = t_p + yw
    c1 = (r1 < 0) & np.isfinite(q1)
    c2 = yw < 0
    q = np.where(
        c1 & c2, np.maximum(q1, q2), np.where(c1, q1, np.where(c2, q2, -np.inf))
    )
    d_ap = 1.0 - t_p
    d_semi = 1.0 - q
    lo = t_p - MARGIN

    # validity from class counts
    _, inv, counts = np.unique(slab, return_inverse=True, return_counts=True)
    cnt_row = counts[inv]
    valid = (cnt_row >= 2) & (cnt_row <= B - 1)

    EDGE = 1e-3
    semi_ok = (c1 | c2) & (q > lo + EDGE) & (q < t_p) & np.isfinite(q)
    redo = valid & ~semi_ok

    per_row = np.where(valid, np.maximum(d_ap - d_semi + MARGIN, 0.0), 0.0)

    if redo.any():
        e = emb / np.maximum(
            np.linalg.norm(emb, axis=1, keepdims=True), 1e-12
        )
        idx = order[np.flatnonzero(redo)]  # original row indices
        for g, i in zip(np.flatnonzero(redo), idx):
            dot = (e[i] @ e.T).astype(np.float32)
            dist = np.clip(1.0 - dot, 0.0, None)
            pos = (lab == lab[i])
            pos[i] = False
            neg = lab != lab[i]
            dap = dist[pos].max()
            semi = neg & (dist > dap) & (dist < dap + MARGIN)
            if semi.any():
                dan = dist[semi].min()
            else:
                dan = dist[neg].min()
            per_row[g] = max(dap - dan + MARGIN, 0.0)

    num_valid = max(int(valid.sum()), 1)
    loss = per_row[valid].sum() / num_valid
    return np.array(loss, dtype=np.float32)


def kernel_run(embeddings, labels, trace=False):
    import concourse.bass_utils as bass_utils

    emb, lab, plan, cores = _prepare(embeddings, labels)
    diag = plan["diag"]
    wins = plan["wins"]
    cp = plan["cp"]
    key = (
        tuple(tuple(d) for d in diag),
        tuple(tuple(w) for w in wins),
        cp,
    )
    if key not in _CACHE:
        _CACHE[key] = _build_bass(diag, wins, cp)
    nc = _CACHE[key]
    in_maps = [
        {"xt": np.ascontiguousarray(c[0]), "xb": np.ascontiguousarray(c[1]),
         "pl": np.ascontiguousarray(c[2]), "pr": np.ascontiguousarray(c[3]),
         "oh": np.ascontiguousarray(c[4])}
        for c in cores
    ]
    res = bass_utils.run_bass_kernel_spmd(
        nc, in_maps, core_ids=list(range(NCORES)), trace=trace
    )
    loss = _host_reduce(emb, lab, plan, res.results)
    return loss, res


def kernel(embeddings, labels):
    loss, _ = kernel_run(embeddings, labels)
    return loss


# revision 27
# speedup vs baseline: 1.1962x; 1.1962x over previous
"""Batch semi-hard triplet loss (cosine distance) on 8 Trainium2 NeuronCores.

Strategy (data-parallel over rows, per sharding hint):
  - Host: sort rows by label; core c takes sorted rows [1024c, 1024(c+1)) in
    8 exact 128-row M-tiles; columns rotated per core so its rows' class
    columns sit in the first PSUM group of each M-tile.
  - Device (per core, uniform SPMD program):
      * normalize embeddings (squares, one-hot column-sum matmuls, sqrt,
        reciprocal, one-hot broadcast matmuls, column scale);
      * per M-tile m: 16 matmuls (4-bank PSUM groups). Class-column poison
        (-2) is applied ON THE PE via small rank-per-class accumulate
        matmuls (lhsT = -2*row-indicators, rhs = col-indicators), so the
        diag group needs no mask adds. t_p (min positive-class dot) comes
        from a narrow poisoned window min on the first group.
        Then three engines split the threshold-max reduction
        q = max{dot < t_p}:
          - Act banks: u = 1/(dot - t_p) (Reciprocal w/ per-partition
            bias), bf16; float min-tree over u -> r1 (min u).
          - Pool banks: y = (dot min t_p) - t_p (one fused gpsimd
            tensor_scalar), bf16: candidates are negative, others +0;
            signed-int16 bit-pattern min over y picks the largest dot
            strictly below t_p (sign bit wraps the threshold).
          - DVE: runs both min-trees (u float-min, y int16-bits-min),
            one M-tile behind.
  - Host: q = max(t_p + 1/r1, t_p + y); per-row loss epilogue in f64;
    rows with no semi-hard candidate in the margin window (or near the
    branch boundary) are recomputed exactly in f32 numpy; mean over valid.
"""

import numpy as np
import ml_dtypes

B = 8192
D = 128
MARGIN = 0.2
NCORES = 8
NT = 512            # N-tile width (one PSUM bank of fp32)
N_NT = B // NT      # 16
MT = 128            # M-tile rows
NMT = B // NCORES // MT  # 8 m-tiles per core
GW = 2              # N-tiles per PSUM group tile (4-deep rotation)
POIS = -2.0         # class-column poison (exactly representable in bf16)

# bank families: Act does recip on banks 2..11; Pool shifts banks 0,1,12..15
ACT_BANKS = list(range(2, 12))
POOL_BANKS = [0, 1, 12, 13, 14, 15]
NA = len(ACT_BANKS)
NP_ = len(POOL_BANKS)

BF16 = ml_dtypes.bfloat16

_CACHE = {}


# --------------------------------------------------------------------------
# host-side planning (pure layout, computed from labels)
# --------------------------------------------------------------------------
def _plan(labels: np.ndarray):
    order = np.argsort(labels, kind="stable")
    slab = labels[order]
    bounds = np.flatnonzero(np.r_[True, slab[1:] != slab[:-1], True])
    cls_start, cls_end = bounds[:-1], bounds[1:]
    row_s = np.empty(B, dtype=np.int64)
    row_e = np.empty(B, dtype=np.int64)
    for s, e in zip(cls_start, cls_end):
        row_s[s:e] = s
        row_e[s:e] = e

    rows_per_core = B // NCORES
    cores = []
    for c in range(NCORES):
        r0 = c * rows_per_core
        base = int(row_s[r0])  # start of first class -> no wraparound
        diag = []
        for m in range(NMT):
            rr = slice(r0 + m * MT, r0 + (m + 1) * MT)
            s = row_s[rr] - base
            e = row_e[rr] - base
            dts = sorted(set((s // NT).tolist()) | set(((e - 1) // NT).tolist()))
            diag.append(dts)
        cores.append(dict(r0=r0, base=base, diag=diag))
    # unify diag sets across cores so all 8 run one compiled program
    uni = [
        sorted(set().union(*[set(pc["diag"][m]) for pc in cores]))
        for m in range(NMT)
    ]
    for pc in cores:
        pc["diag"] = uni
    # per (m, diag tile): narrow column window [c0, c1) within the bank that
    # contains every class column of the tile's rows, across all cores
    wins = []
    for m in range(NMT):
        wm_ = []
        for d in uni[m]:
            c0, c1 = NT, 0
            for pc in cores:
                rr = slice(pc["r0"] + m * MT, pc["r0"] + (m + 1) * MT)
                s = np.maximum(row_s[rr] - pc["base"] - d * NT, 0)
                e = np.minimum(row_e[rr] - pc["base"] - d * NT, NT)
                ok = s < e
                if ok.any():
                    c0 = min(c0, int(s[ok].min()))
                    c1 = max(c1, int(e[ok].max()))
            if c1 <= c0:
                c0, c1 = 0, NT
            wm_.append((c0, c1))
        wins.append(wm_)
    # max classes per (m, diag-tile) block across cores (pois matmul k-dim)
    cp = 1
    for c in range(NCORES):
        pc = cores[c]
        r0, base = pc["r0"], pc["base"]
        for m in range(NMT):
            rr = slice(r0 + m * MT, r0 + (m + 1) * MT)
            ss = row_s[rr]
            for d in uni[m]:
                lo, hi = base + d * NT, base + (d + 1) * NT
                # classes whose column range intersects the bank
                cls = set()
                for g in range(rr.start, rr.stop):
                    if row_s[g] < hi and row_e[g] > lo:
                        cls.add(int(row_s[g]))
                cp = max(cp, len(cls))
    return dict(
        order=order, row_s=row_s, row_e=row_e, cores=cores, diag=uni,
        wins=wins, cp=cp,
    )


def _build_core_inputs(emb_sorted: np.ndarray, plan, c: int):
    """Returns (xt_rot [D,B], xb [D,1024], pl [CP, nblk*MT],
    pr [CP, nblk*NT]) all bf16."""
    pc = plan["cores"][c]
    base, r0 = pc["base"], pc["r0"]
    rows_per_core = B // NCORES
    row_s, row_e = plan["row_s"], plan["row_e"]
    cp = plan["cp"]

    rot = np.r_[np.arange(base, B), np.arange(0, base)]
    xt_rot = np.ascontiguousarray(emb_sorted[rot].T).astype(BF16)
    xb = np.ascontiguousarray(emb_sorted[r0 : r0 + rows_per_core].T).astype(BF16)

    # poison matmul blocks: per (m, d in diag[m]):
    #   pl[k, i] = -2 if m-tile row i in class k else 0     [CP, MT]
    #   pr[k, j] = 1 if bank-d col j in class k else 0      [CP, NT]
    nblk = sum(len(d) for d in pc["diag"])
    pl = np.zeros((cp, nblk * MT), np.float32)
    pr = np.zeros((cp, nblk * NT), np.float32)
    bi = 0
    for m in range(NMT):
        for d in pc["diag"][m]:
            lo, hi = base + d * NT, base + (d + 1) * NT
            cls = {}
            for r in range(MT):
                g = r0 + m * MT + r
                s, e = int(row_s[g]), int(row_e[g])
                if s < hi and e > lo:
                    k = cls.setdefault(s, len(cls))
                    pl[k, bi * MT + r] = POIS
                    cs, ce = max(s - lo, 0), min(e - lo, NT)
                    pr[k, bi * NT + cs : bi * NT + ce] = 1.0
            assert len(cls) <= cp
            bi += 1
    pl = pl.astype(BF16)
    pr = pr.astype(BF16)

    # one-hot routing blocks for the n2 colsum matmuls: batch A (10 tiles)
    # then batch B (8 tiles); block k is [D, NR] with ones in column k
    NRA, NRB = 10, 8
    oh = np.zeros((D, NRA * NRA + NRB * NRB), np.float32)
    for k in range(NRA):
        oh[:, k * NRA + k] = 1.0
    for k in range(NRB):
        oh[:, NRA * NRA + k * NRB + k] = 1.0
    oh = oh.astype(BF16)
    return xt_rot, xb, pl, pr, oh


# --------------------------------------------------------------------------
# device program
# --------------------------------------------------------------------------
def _raw_recip_bias(nc, out, in_, bias_ap):
    import concourse.mybir as mybir

    eng = nc.scalar
    ins = [
        eng.lower_ap(in_),
        eng.lower_ap(bias_ap),
        mybir.ImmediateValue(dtype=mybir.dt.float32, value=1.0),  # scale
        mybir.ImmediateValue(dtype=mybir.dt.float32, value=0.0),  # alpha
    ]
    return eng.add_instruction(
        mybir.InstActivation(
            name=f"I-{nc.next_id()}",
            func=mybir.ActivationFunctionType.Reciprocal,
            ins=ins,
            outs=[eng.lower_ap(out)],
        )
    )


def _build_bass(diag, wins, cp):
    import concourse.bacc as bacc
    import concourse.mybir as mybir
    from concourse.tile import TileContext

    f32 = mybir.dt.float32
    bf16 = mybir.dt.bfloat16
    i16 = mybir.dt.int16
    Alu = mybir.AluOpType
    Act = mybir.ActivationFunctionType
    NOH = N_NT + 2
    NBC = NMT * MT  # xb columns (1024)
    nblk = sum(len(d) for d in diag)

    nc = bacc.Bacc("TRN2", target_bir_lowering=False, debug=False, num_devices=NCORES)

    xt_d = nc.dram_tensor("xt", [D, B], bf16, kind="ExternalInput").ap()
    xb_d = nc.dram_tensor("xb", [D, NBC], bf16, kind="ExternalInput").ap()
    pl_d = nc.dram_tensor("pl", [cp, nblk * MT], bf16, kind="ExternalInput").ap()
    pr_d = nc.dram_tensor("pr", [cp, nblk * NT], bf16, kind="ExternalInput").ap()
    NRA, NRB = 10, 8
    oh_d = nc.dram_tensor(
        "oh", [D, NRA * NRA + NRB * NRB], bf16, kind="ExternalInput"
    ).ap()
    out_d = nc.dram_tensor("out", [MT, 2 * NMT], f32, kind="ExternalOutput").ap()
    outy_d = nc.dram_tensor("outy", [MT, NMT], bf16, kind="ExternalOutput").ap()
    NOH = N_NT + 2
    scr_d = nc.dram_tensor("scr", [1, NOH * NT], bf16, kind="Internal").ap()

    # diag-block flat index per (m, d)
    blkof = {}
    bi = 0
    for m in range(NMT):
        for j, d in enumerate(diag[m]):
            blkof[(m, d)] = bi
            bi += 1

    with TileContext(nc) as tc:
        with (
            tc.tile_pool(name="big", bufs=1) as big,
            tc.tile_pool(name="upool", bufs=2) as upool,
            tc.tile_pool(name="ypool", bufs=2) as ypool,
            tc.tile_pool(name="scr", bufs=2) as scr,
            tc.tile_pool(name="sm", bufs=6) as smp,
            tc.tile_pool(name="psw", bufs=8 // GW, space="PSUM") as psw,
        ):
            # ---------------- setup: load + normalize (pipelined) -----------
            oh = big.tile([D, NRA * NRA + NRB * NRB], bf16, tag="oh")
            nc.sync.dma_start(oh[:], oh_d)
            xb = big.tile([D, NBC], bf16, tag="xb")
            nc.sync.dma_start(xb[:], xb_d)
            pl = big.tile([cp, nblk * MT], bf16, tag="pl")
            nc.sync.dma_start(pl[:], pl_d)
            pr = big.tile([cp, nblk * NT], bf16, tag="pr")
            nc.sync.dma_start(pr[:], pr_d)
            xt = big.tile([D, B], bf16, tag="xt")
            for j in range(8):
                sl = slice(j * (B // 8), (j + 1) * (B // 8))
                nc.sync.dma_start(xt[:, sl], xt_d[:, sl])

            sq = big.tile([D, NOH * NT], bf16, tag="sq")
            xtn = big.tile([D, B], bf16, tag="xtn")
            xbn = big.tile([D, NBC], bf16, tag="xbn")
            rn128 = big.tile([MT, NOH * NT], bf16, tag="rn128")
            outb = big.tile([MT, 2 * NMT], f32, tag="outb")
            outy = big.tile([MT, NMT], bf16, tag="outy")
            ntpall = big.tile([MT, NMT], f32, tag="ntpall")

            # rn128 column layout: tiles [xb0, xb1, xt0..xt15] (NOH slots);
            # batch A = slots 0..9 (xb + xt tiles 0..7), batch B = 10..17.
            def norm_batch(slot0, oh0, chunks):
                """chunks: list of (slot, src_ap, ntiles, engine); squares in
                wide ops; one-hot n2 colsum matmuls accumulate into PSUM rows;
                batched rsqrt + bf16 copy; rearrange + doubling DMAs into
                rn128."""
                nrow = sum(ch[2] for ch in chunks)
                pn = psw.tile([MT, GW * NT], f32, tag="w", name="pn")
                k = 0
                for slot, src, ntiles, eng in chunks:
                    dst = sq[:, slot * NT : (slot + ntiles) * NT]
                    if eng == "act":
                        nc.scalar.activation(dst, src, Act.Square)
                    elif eng == "pool":
                        nc.gpsimd.tensor_tensor(dst, src, src, Alu.mult)
                    else:
                        nc.vector.tensor_tensor(dst, src, src, Alu.mult)
                    for t in range(ntiles):
                        nc.tensor.matmul(
                            pn[0:nrow, :NT],
                            oh[:, oh0 + k * nrow : oh0 + (k + 1) * nrow],
                            sq[:, (slot + t) * NT : (slot + t + 1) * NT],
                            start=(k == 0), stop=(k == nrow - 1),
                        )
                        k += 1
                s0 = smp.tile([NOH, NT], f32, tag="s0", name="s0", bufs=2)
                nc.scalar.activation(s0[0:nrow, :], pn[0:nrow, :NT], Act.Sqrt)
                r0 = smp.tile([NOH, NT], f32, tag="r0", name="r0", bufs=2)
                nc.vector.reciprocal(r0[0:nrow, :], s0[0:nrow, :])
                rb = smp.tile([NOH, NT], bf16, tag="rb", name="rb", bufs=2)
                nc.scalar.copy(rb[0:nrow, :], r0[0:nrow, :])
                c0, c1 = slot0 * NT, (slot0 + nrow) * NT
                # broadcast across partitions via a DRAM round-trip: one
                # flattening store, one zero-partition-step broadcast load
                nc.sync.dma_start(scr_d[0:1, c0:c1], rb[0:nrow, :])
                nc.sync.dma_start(
                    rn128[:, c0:c1],
                    scr_d[0:1, c0:c1].partition_broadcast(MT),
                )

            norm_batch(0, 0, [
                (0, xb[:], 2, "vector"),
                (2, xt[:, 0 : 4 * NT], 4, "act"),
                (6, xt[:, 4 * NT : 8 * NT], 4, "vector"),
            ])
            norm_batch(10, NRA * NRA, [
                (10, xt[:, 8 * NT : 12 * NT], 4, "act"),
                (14, xt[:, 12 * NT : 16 * NT], 4, "pool"),
            ])

            # scales: all-SBUF bf16 (DVE 2x mode)
            nc.vector.tensor_tensor(xbn[:], xb[:], rn128[:, 0 : 2 * NT], Alu.mult)
            for h in range(4):
                sl = slice(h * 4 * NT, (h + 1) * 4 * NT)
                nc.vector.tensor_tensor(
                    xtn[:, sl], xt[:, sl],
                    rn128[:, (2 + 4 * h) * NT : (6 + 4 * h) * NT], Alu.mult,
                )

            # ---------------- main loop over M-tiles ----------------
            # per-bank slot in the u (Act) / y (Pool) buffers
            uslot = {b: i for i, b in enumerate(ACT_BANKS)}
            yslot = {b: i for i, b in enumerate(POOL_BANKS)}

            pending_tree = None  # (u, y, m) of previous M-tile

            def emit_trees(u, y, m):
                # u float min-tree: [MT, NA*NT] bf16 -> r1 -> outb[:, NMT+m]
                w = NA * NT  # 5120
                t1 = scr.tile([MT, w // 2], bf16, tag="ut1")
                nc.vector.tensor_tensor(t1[:], u[:, : w // 2], u[:, w // 2 :], Alu.min)
                t2 = scr.tile([MT, w // 4], bf16, tag="ut2")
                nc.vector.tensor_tensor(
                    t2[:], t1[:, : w // 4], t1[:, w // 4 :], Alu.min
                )
                t3 = scr.tile([MT, w // 8], bf16, tag="ut3")
                nc.vector.tensor_tensor(
                    t3[:], t2[:, : w // 8], t2[:, w // 8 :], Alu.min
                )
                t4 = scr.tile([MT, w // 16], bf16, tag="ut4")
                nc.vector.tensor_tensor(
                    t4[:], t3[:, : w // 16], t3[:, w // 16 :], Alu.min
                )
                t5 = scr.tile([MT, w // 32], bf16, tag="ut5")
                nc.vector.tensor_tensor(
                    t5[:], t4[:, : w // 32], t4[:, w // 32 :], Alu.min
                )
                nc.vector.tensor_reduce(
                    outb[:, NMT + m : NMT + m + 1], t5[:],
                    axis=mybir.AxisListType.X, op=Alu.min,
                )
                # y int16-bits min-tree: [MT, NP_*NT] bf16 -> outy[:, m]
                wy = NP_ * NT  # 3072
                yi = y[:].bitcast(i16)
                s1 = scr.tile([MT, wy // 2], i16, tag="yt1")
                nc.vector.tensor_tensor(
                    s1[:], yi[:, : wy // 2], yi[:, wy // 2 :], Alu.min
                )
                s2 = scr.tile([MT, wy // 4], i16, tag="yt2")
                nc.vector.tensor_tensor(
                    s2[:], s1[:, : wy // 4], s1[:, wy // 4 :], Alu.min
                )
                s3 = scr.tile([MT, wy // 8], i16, tag="yt3")
                nc.vector.tensor_tensor(
                    s3[:], s2[:, : wy // 8], s2[:, wy // 8 :], Alu.min
                )
                s4 = scr.tile([MT, wy // 16], i16, tag="yt4")
                nc.vector.tensor_tensor(
                    s4[:], s3[:, : wy // 16], s3[:, wy // 16 :], Alu.min
                )
                nc.vector.tensor_reduce(
                    outy[:, m : m + 1].bitcast(i16), s4[:],
                    axis=mybir.AxisListType.X, op=Alu.min,
                )

            def emit_tp(m):
                """Narrow-window diag matmuls (+pois) into one PSUM tile,
                then the t_p chain: Pool window mins -> DVE finalize."""
                dts = diag[m]
                lhsT = xbn[:, m * MT : (m + 1) * MT]
                tpp = outb[:, m : m + 1]
                ntp = ntpall[:, m : m + 1]
                minis = {}
                for j, d in enumerate(dts):
                    if j % GW == 0:
                        mini = psw.tile([MT, GW * NT], f32, tag="w")
                        minis[j // GW] = mini
                    jj = j % GW
                    c0, c1 = wins[m][j]
                    w = c1 - c0
                    i = blkof[(m, d)]
                    nc.tensor.matmul(
                        mini[:, jj * NT : jj * NT + w],
                        lhsT, xtn[:, d * NT + c0 : d * NT + c1],
                        start=True, stop=False,
                    )
                    nc.tensor.matmul(
                        mini[:, jj * NT : jj * NT + w],
                        pl[:, i * MT : (i + 1) * MT],
                        pr[:, i * NT + c0 : i * NT + c1],
                        start=False, stop=True,
                    )
                ndts = len(dts)
                posm = smp.tile([MT, max(ndts, 1)], f32, tag="posm")
                for j, d in enumerate(dts):
                    c0, c1 = wins[m][j]
                    jj = j % GW
                    nc.vector.tensor_reduce(
                        posm[:, j : j + 1],
                        minis[j // GW][:, jj * NT : jj * NT + (c1 - c0)],
                        axis=mybir.AxisListType.X, op=Alu.min,
                    )
                if ndts == 1:
                    minpos = posm[:, 0:1]
                else:
                    mp = smp.tile([MT, 1], f32, tag="minpos")
                    nc.vector.tensor_reduce(
                        mp[:], posm[:], axis=mybir.AxisListType.X, op=Alu.min
                    )
                    minpos = mp[:]
                # t_p = min(minpos - POIS, 1)
                nc.vector.tensor_scalar(
                    tpp, minpos, -POIS, 1.0, Alu.add, Alu.min
                )
                nc.vector.tensor_scalar_mul(ntp, tpp, -1.0)

            emit_tp(0)

            for m in range(NMT):
                dts = diag[m]
                lhsT = xbn[:, m * MT : (m + 1) * MT]
                tpp = outb[:, m : m + 1]       # +t_p ptr (f32)
                ntp = ntpall[:, m : m + 1]     # -t_p ptr (f32)
                u = upool.tile([MT, NA * NT], bf16, tag="u")
                y = ypool.tile([MT, NP_ * NT], bf16, tag="y")

                for g in range(N_NT // GW):
                    wg = psw.tile([MT, GW * NT], f32, tag="w")
                    for k in range(GW):
                        t = GW * g + k
                        if t in dts:
                            nc.tensor.matmul(
                                wg[:, k * NT : (k + 1) * NT],
                                lhsT, xtn[:, t * NT : (t + 1) * NT],
                                start=True, stop=False,
                            )
                            i = blkof[(m, t)]
                            nc.tensor.matmul(
                                wg[:, k * NT : (k + 1) * NT],
                                pl[:, i * MT : (i + 1) * MT],
                                pr[:, i * NT : (i + 1) * NT],
                                start=False, stop=True,
                            )
                        else:
                            nc.tensor.matmul(
                                wg[:, k * NT : (k + 1) * NT],
                                lhsT, xtn[:, t * NT : (t + 1) * NT],
                            )

                    # consumers (t_p for this m was precomputed)
                    bank0 = GW * g
                    ab = [b for b in range(bank0, bank0 + GW) if b in uslot]
                    if ab:
                        k0, k1 = ab[0] - bank0, ab[-1] - bank0 + 1
                        _raw_recip_bias(
                            nc,
                            u[:, uslot[ab[0]] * NT : uslot[ab[-1]] * NT + NT],
                            wg[:, k0 * NT : k1 * NT],
                            ntp,
                        )
                    pb = [b for b in range(bank0, bank0 + GW) if b in yslot]
                    if pb:
                        k0, k1 = pb[0] - bank0, pb[-1] - bank0 + 1
                        nc.gpsimd.tensor_scalar(
                            y[:, yslot[pb[0]] * NT : yslot[pb[-1]] * NT + NT],
                            wg[:, k0 * NT : k1 * NT],
                            tpp, tpp, Alu.min, Alu.subtract,
                        )

                # next M-tile's t_p runs on PE/Pool/DVE while this one's
                # Act/Pool streams are still draining
                if m + 1 < NMT:
                    emit_tp(m + 1)
                if pending_tree is not None:
                    emit_trees(*pending_tree)
                pending_tree = (u, y, m)

            emit_trees(*pending_tree)

            nc.sync.dma_start(out_d, outb[:])
            nc.sync.dma_start(outy_d, outy[:])

    nc.compile()
    return nc


# --------------------------------------------------------------------------
# entry point
# --------------------------------------------------------------------------
def _prepare(embeddings, labels):
    emb = np.asarray(embeddings, dtype=np.float32)
    lab = np.asarray(labels).astype(np.int64)
    plan = _plan(lab)
    emb_sorted = emb[plan["order"]]
    cores = [_build_core_inputs(emb_sorted, plan, c) for c in range(NCORES)]
    return emb, lab, plan, cores


def _host_reduce(emb, lab, plan, outs):
    """outs: per core {"out": [128, 16] f32, "outy": [128, 8] bf16}."""
    order = plan["order"]
    slab = lab[order]
    rows_per_core = B // NCORES

    t_p = np.zeros(B, np.float64)
    r1 = np.zeros(B, np.float64)
    yw = np.zeros(B, np.float64)
    for c in range(NCORES):
        o = np.asarray(outs[c]["out"], np.float64)
        oy = np.asarray(outs[c]["outy"]).astype(np.float64)
        for m in range(NMT):
            rr = slice(c * rows_per_core + m * MT, c * rows_per_core + (m + 1) * MT)
            t_p[rr] = o[:, m]
            r1[rr] = o[:, NMT + m]
            yw[rr] = oy[:, m]

    with np.errstate(divide="ignore", invalid="ignore"):
        q1 = t_p + 1.0 / r1
    q2 = t_p + yw
    c1 = (r1 < 0) & np.isfinite(q1)
    c2 = yw < 0
    q = np.where(
        c1 & c2, np.maximum(q1, q2), np.where(c1, q1, np.where(c2, q2, -np.inf))
    )
    d_ap = 1.0 - t_p
    d_semi = 1.0 - q
    lo = t_p - MARGIN

    # validity from class counts
    _, inv, counts = np.unique(slab, return_inverse=True, return_counts=True)
    cnt_row = counts[inv]
    valid = (cnt_row >= 2) & (cnt_row <= B - 1)

    EDGE = 1e-3
    semi_ok = (c1 | c2) & (q > lo + EDGE) & (q < t_p) & np.isfinite(q)
    redo = valid & ~semi_ok

    per_row = np.where(valid, np.maximum(d_ap - d_semi + MARGIN, 0.0), 0.0)

    if redo.any():
        e = emb / np.maximum(
            np.linalg.norm(emb, axis=1, keepdims=True), 1e-12
        )
        idx = order[np.flatnonzero(redo)]  # original row indices
        for g, i in zip(np.flatnonzero(redo), idx):
            dot = (e[i] @ e.T).astype(np.float32)
            dist = np.clip(1.0 - dot, 0.0, None)
            pos = (lab == lab[i])
            pos[i] = False
            neg = lab != lab[i]
            dap = dist[pos].max()
            semi = neg & (dist > dap) & (dist < dap + MARGIN)
            if semi.any():
                dan = dist[semi].min()
            else:
                dan = dist[neg].min()
            per_row[g] = max(dap - dan + MARGIN, 0.0)

    num_valid = max(int(valid.sum()), 1)
    loss = per_row[valid].sum() / num_valid
    return np.array(loss, dtype=np.float32)


def kernel_run(embeddings, labels, trace=False):
    import concourse.bass_utils as bass_utils

    emb, lab, plan, cores = _prepare(embeddings, labels)
    diag = plan["diag"]
    wins = plan["wins"]
    cp = plan["cp"]
    key = (
        tuple(tuple(d) for d in diag),
        tuple(tuple(w) for w in wins),
        cp,
    )
    if key not in _CACHE:
        _CACHE[key] = _build_bass(diag, wins, cp)
    nc = _CACHE[key]
    in_maps = [
        {"xt": np.ascontiguousarray(c[0]), "xb": np.ascontiguousarray(c[1]),
         "pl": np.ascontiguousarray(c[2]), "pr": np.ascontiguousarray(c[3]),
         "oh": np.ascontiguousarray(c[4])}
        for c in cores
    ]
    res = bass_utils.run_bass_kernel_spmd(
        nc, in_maps, core_ids=list(range(NCORES)), trace=trace
    )
    loss = _host_reduce(emb, lab, plan, res.results)
    return loss, res


def kernel(embeddings, labels):
    loss, _ = kernel_run(embeddings, labels)
    return loss


# revision 33
# speedup vs baseline: 1.5491x; 1.2951x over previous
"""Batch semi-hard triplet loss (cosine distance) on 8 Trainium2 NeuronCores.

Strategy (data-parallel over rows, per sharding hint):
  - Host: sort rows by label; core c takes sorted rows [1024c, 1024(c+1)) in
    8 exact 128-row M-tiles; columns rotated per core so its rows' class
    columns sit in the first PSUM group of each M-tile.
  - Device (per core, uniform SPMD program):
      * normalize embeddings (squares, one-hot column-sum matmuls, sqrt,
        reciprocal, one-hot broadcast matmuls, column scale);
      * per M-tile m: 16 matmuls (4-bank PSUM groups). Class-column poison
        (-2) is applied ON THE PE via small rank-per-class accumulate
        matmuls (lhsT = -2*row-indicators, rhs = col-indicators), so the
        diag group needs no mask adds. t_p (min positive-class dot) comes
        from a narrow poisoned window min on the first group.
        Then three engines split the threshold-max reduction
        q = max{dot < t_p}:
          - Act banks: u = 1/(dot - t_p) (Reciprocal w/ per-partition
            bias), bf16; float min-tree over u -> r1 (min u).
          - Pool banks: y = (dot min t_p) - t_p (one fused gpsimd
            tensor_scalar), bf16: candidates are negative, others +0;
            signed-int16 bit-pattern min over y picks the largest dot
            strictly below t_p (sign bit wraps the threshold).
          - DVE: runs both min-trees (u float-min, y int16-bits-min),
            one M-tile behind.
  - Host: q = max(t_p + 1/r1, t_p + y); per-row loss epilogue in f64;
    rows with no semi-hard candidate in the margin window (or near the
    branch boundary) are recomputed exactly in f32 numpy; mean over valid.
"""

import numpy as np
import ml_dtypes

B = 8192
D = 128
MARGIN = 0.2
NCORES = 8
NT = 512            # N-tile width (one PSUM bank of fp32)
N_NT = B // NT      # 16
MT = 128            # M-tile rows
NMT = B // NCORES // MT  # 8 m-tiles per core
GW = 2              # N-tiles per PSUM group tile (4-deep rotation)
POIS = -2.0         # class-column poison (exactly representable in bf16)

# bank families: Act does recip on banks 2..11; Pool shifts banks 0,1,12..15
ACT_BANKS = list(range(2, 12))
POOL_BANKS = [0, 1, 12, 13, 14, 15]
NA = len(ACT_BANKS)
NP_ = len(POOL_BANKS)

BF16 = ml_dtypes.bfloat16

_CACHE = {}


# --------------------------------------------------------------------------
# host-side planning (pure layout, computed from labels)
# --------------------------------------------------------------------------
def _plan(labels: np.ndarray):
    order = np.argsort(labels, kind="stable")
    slab = labels[order]
    bounds = np.flatnonzero(np.r_[True, slab[1:] != slab[:-1], True])
    cls_start, cls_end = bounds[:-1], bounds[1:]
    row_s = np.empty(B, dtype=np.int64)
    row_e = np.empty(B, dtype=np.int64)
    for s, e in zip(cls_start, cls_end):
        row_s[s:e] = s
        row_e[s:e] = e

    rows_per_core = B // NCORES
    cores = []
    for c in range(NCORES):
        r0 = c * rows_per_core
        base = int(row_s[r0])  # start of first class -> no wraparound
        diag = []
        for m in range(NMT):
            rr = slice(r0 + m * MT, r0 + (m + 1) * MT)
            s = row_s[rr] - base
            e = row_e[rr] - base
            dts = sorted(set((s // NT).tolist()) | set(((e - 1) // NT).tolist()))
            diag.append(dts)
        cores.append(dict(r0=r0, base=base, diag=diag))
    # unify diag sets across cores so all 8 run one compiled program
    uni = [
        sorted(set().union(*[set(pc["diag"][m]) for pc in cores]))
        for m in range(NMT)
    ]
    for pc in cores:
        pc["diag"] = uni
    # per (m, diag tile): narrow column window [c0, c1) within the bank that
    # contains every class column of the tile's rows, across all cores
    wins = []
    for m in range(NMT):
        wm_ = []
        for d in uni[m]:
            c0, c1 = NT, 0
            for pc in cores:
                rr = slice(pc["r0"] + m * MT, pc["r0"] + (m + 1) * MT)
                s = np.maximum(row_s[rr] - pc["base"] - d * NT, 0)
                e = np.minimum(row_e[rr] - pc["base"] - d * NT, NT)
                ok = s < e
                if ok.any():
                    c0 = min(c0, int(s[ok].min()))
                    c1 = max(c1, int(e[ok].max()))
            if c1 <= c0:
                c0, c1 = 0, NT
            wm_.append((c0, c1))
        wins.append(wm_)
    # max classes per (m, diag-tile) block across cores (pois matmul k-dim)
    cp = 1
    for c in range(NCORES):
        pc = cores[c]
        r0, base = pc["r0"], pc["base"]
        for m in range(NMT):
            rr = slice(r0 + m * MT, r0 + (m + 1) * MT)
            ss = row_s[rr]
            for d in uni[m]:
                lo, hi = base + d * NT, base + (d + 1) * NT
                # classes whose column range intersects the bank
                cls = set()
                for g in range(rr.start, rr.stop):
                    if row_s[g] < hi and row_e[g] > lo:
                        cls.add(int(row_s[g]))
                cp = max(cp, len(cls))
    return dict(
        order=order, row_s=row_s, row_e=row_e, cores=cores, diag=uni,
        wins=wins, cp=cp,
    )


def _build_core_inputs(emb_norm: np.ndarray, plan, c: int):
    """emb_norm: label-sorted, unit-normalized embeddings (f32).
    Returns (xtn_rot [D,B], xbn [D,1024], pl [CP, nblk*MT],
    pr [CP, nblk*NT]) all bf16."""
    pc = plan["cores"][c]
    base, r0 = pc["base"], pc["r0"]
    rows_per_core = B // NCORES
    row_s, row_e = plan["row_s"], plan["row_e"]
    cp = plan["cp"]

    rot = np.r_[np.arange(base, B), np.arange(0, base)]
    xt_rot = np.ascontiguousarray(emb_norm[rot].T).astype(BF16)
    xb = np.ascontiguousarray(emb_norm[r0 : r0 + rows_per_core].T).astype(BF16)

    # poison matmul blocks: per (m, d in diag[m]):
    #   pl[k, i] = -2 if m-tile row i in class k else 0     [CP, MT]
    #   pr[k, j] = 1 if bank-d col j in class k else 0      [CP, NT]
    nblk = sum(len(d) for d in pc["diag"])
    pl = np.zeros((cp, nblk * MT), np.float32)
    pr = np.zeros((cp, nblk * NT), np.float32)
    bi = 0
    for m in range(NMT):
        for d in pc["diag"][m]:
            lo, hi = base + d * NT, base + (d + 1) * NT
            cls = {}
            for r in range(MT):
                g = r0 + m * MT + r
                s, e = int(row_s[g]), int(row_e[g])
                if s < hi and e > lo:
                    k = cls.setdefault(s, len(cls))
                    pl[k, bi * MT + r] = POIS
                    cs, ce = max(s - lo, 0), min(e - lo, NT)
                    pr[k, bi * NT + cs : bi * NT + ce] = 1.0
            assert len(cls) <= cp
            bi += 1
    pl = pl.astype(BF16)
    pr = pr.astype(BF16)

    return xt_rot, xb, pl, pr


# --------------------------------------------------------------------------
# device program
# --------------------------------------------------------------------------
def _raw_recip_bias(nc, out, in_, bias_ap):
    import concourse.mybir as mybir

    eng = nc.scalar
    ins = [
        eng.lower_ap(in_),
        eng.lower_ap(bias_ap),
        mybir.ImmediateValue(dtype=mybir.dt.float32, value=1.0),  # scale
        mybir.ImmediateValue(dtype=mybir.dt.float32, value=0.0),  # alpha
    ]
    return eng.add_instruction(
        mybir.InstActivation(
            name=f"I-{nc.next_id()}",
            func=mybir.ActivationFunctionType.Reciprocal,
            ins=ins,
            outs=[eng.lower_ap(out)],
        )
    )


def _build_bass(diag, wins, cp):
    import concourse.bacc as bacc
    import concourse.mybir as mybir
    from concourse.tile import TileContext

    f32 = mybir.dt.float32
    bf16 = mybir.dt.bfloat16
    i16 = mybir.dt.int16
    Alu = mybir.AluOpType
    Act = mybir.ActivationFunctionType
    NOH = N_NT + 2
    NBC = NMT * MT  # xb columns (1024)
    nblk = sum(len(d) for d in diag)

    nc = bacc.Bacc("TRN2", target_bir_lowering=False, debug=False, num_devices=NCORES)

    xt_d = nc.dram_tensor("xt", [D, B], bf16, kind="ExternalInput").ap()
    xb_d = nc.dram_tensor("xb", [D, NBC], bf16, kind="ExternalInput").ap()
    pl_d = nc.dram_tensor("pl", [cp, nblk * MT], bf16, kind="ExternalInput").ap()
    pr_d = nc.dram_tensor("pr", [cp, nblk * NT], bf16, kind="ExternalInput").ap()
    out_d = nc.dram_tensor("out", [MT, 2 * NMT], f32, kind="ExternalOutput").ap()
    outy_d = nc.dram_tensor("outy", [MT, NMT], bf16, kind="ExternalOutput").ap()

    # diag-block flat index per (m, d)
    blkof = {}
    bi = 0
    for m in range(NMT):
        for j, d in enumerate(diag[m]):
            blkof[(m, d)] = bi
            bi += 1

    with TileContext(nc) as tc:
        with (
            tc.tile_pool(name="big", bufs=1) as big,
            tc.tile_pool(name="upool", bufs=2) as upool,
            tc.tile_pool(name="ypool", bufs=2) as ypool,
            tc.tile_pool(name="scr", bufs=2) as scr,
            tc.tile_pool(name="sm", bufs=6) as smp,
            tc.tile_pool(name="psw", bufs=8 // GW, space="PSUM") as psw,
        ):
            # ---------------- setup: load pre-normalized inputs -------------
            # xbn + first xtn chunks on the SP HWDGE queue; the back half of
            # xtn on the Activation HWDGE queue (two queues load in parallel)
            xbn = big.tile([D, NBC], bf16, tag="xbn")
            nc.sync.dma_start(xbn[:], xb_d)
            pl = big.tile([cp, nblk * MT], bf16, tag="pl")
            nc.sync.dma_start(pl[:], pl_d)
            pr = big.tile([cp, nblk * NT], bf16, tag="pr")
            nc.sync.dma_start(pr[:], pr_d)
            xtn = big.tile([D, B], bf16, tag="xtn")
            for j in range(4):
                sl = slice(j * (B // 8), (j + 1) * (B // 8))
                nc.sync.dma_start(xtn[:, sl], xt_d[:, sl])
            for j in range(4, 8):
                sl = slice(j * (B // 8), (j + 1) * (B // 8))
                nc.scalar.dma_start(xtn[:, sl], xt_d[:, sl])

            outb = big.tile([MT, 2 * NMT], f32, tag="outb")
            outy = big.tile([MT, NMT], bf16, tag="outy")
            ntpall = big.tile([MT, NMT], f32, tag="ntpall")

            # ---------------- main loop over M-tiles ----------------
            # per-bank slot in the u (Act) / y (Pool) buffers
            uslot = {b: i for i, b in enumerate(ACT_BANKS)}
            yslot = {b: i for i, b in enumerate(POOL_BANKS)}

            pending_tree = None  # (u, y, m) of previous M-tile

            def emit_trees(u, y, m):
                # u float min-tree: [MT, NA*NT] bf16 -> r1 -> outb[:, NMT+m]
                w = NA * NT  # 5120
                t1 = scr.tile([MT, w // 2], bf16, tag="ut1")
                nc.vector.tensor_tensor(t1[:], u[:, : w // 2], u[:, w // 2 :], Alu.min)
                t2 = scr.tile([MT, w // 4], bf16, tag="ut2")
                nc.vector.tensor_tensor(
                    t2[:], t1[:, : w // 4], t1[:, w // 4 :], Alu.min
                )
                t3 = scr.tile([MT, w // 8], bf16, tag="ut3")
                nc.vector.tensor_tensor(
                    t3[:], t2[:, : w // 8], t2[:, w // 8 :], Alu.min
                )
                t4 = scr.tile([MT, w // 16], bf16, tag="ut4")
                nc.vector.tensor_tensor(
                    t4[:], t3[:, : w // 16], t3[:, w // 16 :], Alu.min
                )
                t5 = scr.tile([MT, w // 32], bf16, tag="ut5")
                nc.vector.tensor_tensor(
                    t5[:], t4[:, : w // 32], t4[:, w // 32 :], Alu.min
                )
                nc.vector.tensor_reduce(
                    outb[:, NMT + m : NMT + m + 1], t5[:],
                    axis=mybir.AxisListType.X, op=Alu.min,
                )
                # y int16-bits min-tree: [MT, NP_*NT] bf16 -> outy[:, m]
                wy = NP_ * NT  # 3072
                yi = y[:].bitcast(i16)
                s1 = scr.tile([MT, wy // 2], i16, tag="yt1")
                nc.vector.tensor_tensor(
                    s1[:], yi[:, : wy // 2], yi[:, wy // 2 :], Alu.min
                )
                s2 = scr.tile([MT, wy // 4], i16, tag="yt2")
                nc.vector.tensor_tensor(
                    s2[:], s1[:, : wy // 4], s1[:, wy // 4 :], Alu.min
                )
                s3 = scr.tile([MT, wy // 8], i16, tag="yt3")
                nc.vector.tensor_tensor(
                    s3[:], s2[:, : wy // 8], s2[:, wy // 8 :], Alu.min
                )
                s4 = scr.tile([MT, wy // 16], i16, tag="yt4")
                nc.vector.tensor_tensor(
                    s4[:], s3[:, : wy // 16], s3[:, wy // 16 :], Alu.min
                )
                nc.vector.tensor_reduce(
                    outy[:, m : m + 1].bitcast(i16), s4[:],
                    axis=mybir.AxisListType.X, op=Alu.min,
                )

            def emit_tp(m):
                """Narrow-window diag matmuls (+pois) into one PSUM tile,
                then the t_p chain: Pool window mins -> DVE finalize."""
                dts = diag[m]
                lhsT = xbn[:, m * MT : (m + 1) * MT]
                tpp = outb[:, m : m + 1]
                ntp = ntpall[:, m : m + 1]
                minis = {}
                for j, d in enumerate(dts):
                    if j % GW == 0:
                        mini = psw.tile([MT, GW * NT], f32, tag="w")
                        minis[j // GW] = mini
                    jj = j % GW
                    c0, c1 = wins[m][j]
                    w = c1 - c0
                    i = blkof[(m, d)]
                    nc.tensor.matmul(
                        mini[:, jj * NT : jj * NT + w],
                        lhsT, xtn[:, d * NT + c0 : d * NT + c1],
                        start=True, stop=False,
                    )
                    nc.tensor.matmul(
                        mini[:, jj * NT : jj * NT + w],
                        pl[:, i * MT : (i + 1) * MT],
                        pr[:, i * NT + c0 : i * NT + c1],
                        start=False, stop=True,
                    )
                ndts = len(dts)
                posm = smp.tile([MT, max(ndts, 1)], f32, tag="posm")
                for j, d in enumerate(dts):
                    c0, c1 = wins[m][j]
                    jj = j % GW
                    nc.vector.tensor_reduce(
                        posm[:, j : j + 1],
                        minis[j // GW][:, jj * NT : jj * NT + (c1 - c0)],
                        axis=mybir.AxisListType.X, op=Alu.min,
                    )
                if ndts == 1:
                    minpos = posm[:, 0:1]
                else:
                    mp = smp.tile([MT, 1], f32, tag="minpos")
                    nc.vector.tensor_reduce(
                        mp[:], posm[:], axis=mybir.AxisListType.X, op=Alu.min
                    )
                    minpos = mp[:]
                # t_p = min(minpos - POIS, 1)
                nc.vector.tensor_scalar(
                    tpp, minpos, -POIS, 1.0, Alu.add, Alu.min
                )
                nc.vector.tensor_scalar_mul(ntp, tpp, -1.0)

            emit_tp(0)

            for m in range(NMT):
                dts = diag[m]
                lhsT = xbn[:, m * MT : (m + 1) * MT]
                tpp = outb[:, m : m + 1]       # +t_p ptr (f32)
                ntp = ntpall[:, m : m + 1]     # -t_p ptr (f32)
                u = upool.tile([MT, NA * NT], bf16, tag="u")
                y = ypool.tile([MT, NP_ * NT], bf16, tag="y")

                for g in range(N_NT // GW):
                    wg = psw.tile([MT, GW * NT], f32, tag="w")
                    for k in range(GW):
                        t = GW * g + k
                        if t in dts:
                            nc.tensor.matmul(
                                wg[:, k * NT : (k + 1) * NT],
                                lhsT, xtn[:, t * NT : (t + 1) * NT],
                                start=True, stop=False,
                            )
                            i = blkof[(m, t)]
                            nc.tensor.matmul(
                                wg[:, k * NT : (k + 1) * NT],
                                pl[:, i * MT : (i + 1) * MT],
                                pr[:, i * NT : (i + 1) * NT],
                                start=False, stop=True,
                            )
                        else:
                            nc.tensor.matmul(
                                wg[:, k * NT : (k + 1) * NT],
                                lhsT, xtn[:, t * NT : (t + 1) * NT],
                            )

                    # consumers (t_p for this m was precomputed)
                    bank0 = GW * g
                    ab = [b for b in range(bank0, bank0 + GW) if b in uslot]
                    if ab:
                        k0, k1 = ab[0] - bank0, ab[-1] - bank0 + 1
                        _raw_recip_bias(
                            nc,
                            u[:, uslot[ab[0]] * NT : uslot[ab[-1]] * NT + NT],
                            wg[:, k0 * NT : k1 * NT],
                            ntp,
                        )
                    pb = [b for b in range(bank0, bank0 + GW) if b in yslot]
                    if pb:
                        k0, k1 = pb[0] - bank0, pb[-1] - bank0 + 1
                        nc.gpsimd.tensor_scalar(
                            y[:, yslot[pb[0]] * NT : yslot[pb[-1]] * NT + NT],
                            wg[:, k0 * NT : k1 * NT],
                            tpp, tpp, Alu.min, Alu.subtract,
                        )

                # next M-tile's t_p runs on PE/Pool/DVE while this one's
                # Act/Pool streams are still draining
                if m + 1 < NMT:
                    emit_tp(m + 1)
                if pending_tree is not None:
                    emit_trees(*pending_tree)
                pending_tree = (u, y, m)

            emit_trees(*pending_tree)

            nc.sync.dma_start(out_d, outb[:])
            nc.sync.dma_start(outy_d, outy[:])

    nc.compile()
    return nc


# --------------------------------------------------------------------------
# entry point
# --------------------------------------------------------------------------
def _prepare(embeddings, labels):
    emb = np.asarray(embeddings, dtype=np.float32)
    lab = np.asarray(labels).astype(np.int64)
    plan = _plan(lab)
    emb_sorted = emb[plan["order"]]
    norm = np.linalg.norm(emb_sorted, axis=1, keepdims=True)
    emb_norm = emb_sorted / np.maximum(norm, 1e-12)
    cores = [_build_core_inputs(emb_norm, plan, c) for c in range(NCORES)]
    return emb, lab, plan, cores


def _host_reduce(emb, lab, plan, outs):
    """outs: per core {"out": [128, 16] f32, "outy": [128, 8] bf16}."""
    order = plan["order"]
    slab = lab[order]
    rows_per_core = B // NCORES

    t_p = np.zeros(B, np.float64)
    r1 = np.zeros(B, np.float64)
    yw = np.zeros(B, np.float64)
    for c in range(NCORES):
        o = np.asarray(outs[c]["out"], np.float64)
        oy = np.asarray(outs[c]["outy"]).astype(np.float64)
        for m in range(NMT):
            rr = slice(c * rows_per_core + m * MT, c * rows_per_core + (m + 1) * MT)
            t_p[rr] = o[:, m]
            r1[rr] = o[:, NMT + m]
            yw[rr] = oy[:, m]

    with np.errstate(divide="ignore", invalid="ignore"):
        q1 = t_p + 1.0 / r1
    q2 = t_p + yw
    c1 = (r1 < 0) & np.isfinite(q1)
    c2 = yw < 0
    q = np.where(
        c1 & c2, np.maximum(q1, q2), np.where(c1, q1, np.where(c2, q2, -np.inf))
    )
    d_ap = 1.0 - t_p
    d_semi = 1.0 - q
    lo = t_p - MARGIN

    # validity from class counts
    _, inv, counts = np.unique(slab, return_inverse=True, return_counts=True)
    cnt_row = counts[inv]
    valid = (cnt_row >= 2) & (cnt_row <= B - 1)

    EDGE = 1e-3
    semi_ok = (c1 | c2) & (q > lo + EDGE) & (q < t_p) & np.isfinite(q)
    redo = valid & ~semi_ok

    per_row = np.where(valid, np.maximum(d_ap - d_semi + MARGIN, 0.0), 0.0)

    if redo.any():
        e = emb / np.maximum(
            np.linalg.norm(emb, axis=1, keepdims=True), 1e-12
        )
        idx = order[np.flatnonzero(redo)]  # original row indices
        for g, i in zip(np.flatnonzero(redo), idx):
            dot = (e[i] @ e.T).astype(np.float32)
            dist = np.clip(1.0 - dot, 0.0, None)
            pos = (lab == lab[i])
            pos[i] = False
            neg = lab != lab[i]
            dap = dist[pos].max()
            semi = neg & (dist > dap) & (dist < dap + MARGIN)
            if semi.any():
                dan = dist[semi].min()
            else:
                dan = dist[neg].min()
            per_row[g] = max(dap - dan + MARGIN, 0.0)

    num_valid = max(int(valid.sum()), 1)
    loss = per_row[valid].sum() / num_valid
    return np.array(loss, dtype=np.float32)


def kernel_run(embeddings, labels, trace=False):
    import concourse.bass_utils as bass_utils

    emb, lab, plan, cores = _prepare(embeddings, labels)
    diag = plan["diag"]
    wins = plan["wins"]
    cp = plan["cp"]
    key = (
        tuple(tuple(d) for d in diag),
        tuple(tuple(w) for w in wins),
        cp,
    )
    if key not in _CACHE:
        _CACHE[key] = _build_bass(diag, wins, cp)
    nc = _CACHE[key]
    in_maps = [
        {"xt": np.ascontiguousarray(c[0]), "xb": np.ascontiguousarray(c[1]),
         "pl": np.ascontiguousarray(c[2]), "pr": np.ascontiguousarray(c[3])}
        for c in cores
    ]
    res = bass_utils.run_bass_kernel_spmd(
        nc, in_maps, core_ids=list(range(NCORES)), trace=trace
    )
    loss = _host_reduce(emb, lab, plan, res.results)
    return loss, res


def kernel(embeddings, labels):
    loss, _ = kernel_run(embeddings, labels)
    return loss


# revision 35
# speedup vs baseline: 1.6243x; 1.0486x over previous
"""Batch semi-hard triplet loss (cosine distance) on 8 Trainium2 NeuronCores.

Strategy (data-parallel over rows, per sharding hint):
  - Host: sort rows by label; core c takes sorted rows [1024c, 1024(c+1)) in
    8 exact 128-row M-tiles; columns rotated per core so its rows' class
    columns sit in the first PSUM group of each M-tile.
  - Device (per core, uniform SPMD program):
      * normalize embeddings (squares, one-hot column-sum matmuls, sqrt,
        reciprocal, one-hot broadcast matmuls, column scale);
      * per M-tile m: 16 matmuls (4-bank PSUM groups). Class-column poison
        (-2) is applied ON THE PE via small rank-per-class accumulate
        matmuls (lhsT = -2*row-indicators, rhs = col-indicators), so the
        diag group needs no mask adds. t_p (min positive-class dot) comes
        from a narrow poisoned window min on the first group.
        Then three engines split the threshold-max reduction
        q = max{dot < t_p}:
          - Act banks: u = 1/(dot - t_p) (Reciprocal w/ per-partition
            bias), bf16; float min-tree over u -> r1 (min u).
          - Pool banks: y = (dot min t_p) - t_p (one fused gpsimd
            tensor_scalar), bf16: candidates are negative, others +0;
            signed-int16 bit-pattern min over y picks the largest dot
            strictly below t_p (sign bit wraps the threshold).
          - DVE: runs both min-trees (u float-min, y int16-bits-min),
            one M-tile behind.
  - Host: q = max(t_p + 1/r1, t_p + y); per-row loss epilogue in f64;
    rows with no semi-hard candidate in the margin window (or near the
    branch boundary) are recomputed exactly in f32 numpy; mean over valid.
"""

import numpy as np
import ml_dtypes

B = 8192
D = 128
MARGIN = 0.2
NCORES = 8
NT = 512            # N-tile width (one PSUM bank of fp32)
N_NT = B // NT      # 16
MT = 128            # M-tile rows
NMT = B // NCORES // MT  # 8 m-tiles per core
GW = 2              # N-tiles per PSUM group tile (4-deep rotation)
POIS = -2.0         # class-column poison (exactly representable in bf16)

# bank families: Act does recip on banks 2..11; Pool shifts banks 0,1,12..15
ACT_BANKS = list(range(2, 12))
POOL_BANKS = [0, 1, 12, 13, 14, 15]
NA = len(ACT_BANKS)
NP_ = len(POOL_BANKS)

BF16 = ml_dtypes.bfloat16

_CACHE = {}


# --------------------------------------------------------------------------
# host-side planning (pure layout, computed from labels)
# --------------------------------------------------------------------------
def _plan(labels: np.ndarray):
    order = np.argsort(labels, kind="stable")
    slab = labels[order]
    bounds = np.flatnonzero(np.r_[True, slab[1:] != slab[:-1], True])
    cls_start, cls_end = bounds[:-1], bounds[1:]
    row_s = np.empty(B, dtype=np.int64)
    row_e = np.empty(B, dtype=np.int64)
    for s, e in zip(cls_start, cls_end):
        row_s[s:e] = s
        row_e[s:e] = e

    rows_per_core = B // NCORES
    cores = []
    for c in range(NCORES):
        r0 = c * rows_per_core
        base = int(row_s[r0])  # start of first class -> no wraparound
        diag = []
        for m in range(NMT):
            rr = slice(r0 + m * MT, r0 + (m + 1) * MT)
            s = row_s[rr] - base
            e = row_e[rr] - base
            dts = sorted(set((s // NT).tolist()) | set(((e - 1) // NT).tolist()))
            diag.append(dts)
        cores.append(dict(r0=r0, base=base, diag=diag))
    # unify diag sets across cores so all 8 run one compiled program
    uni = [
        sorted(set().union(*[set(pc["diag"][m]) for pc in cores]))
        for m in range(NMT)
    ]
    for pc in cores:
        pc["diag"] = uni
    # per (m, diag tile): narrow column window [c0, c1) within the bank that
    # contains every class column of the tile's rows, across all cores
    wins = []
    for m in range(NMT):
        wm_ = []
        for d in uni[m]:
            c0, c1 = NT, 0
            for pc in cores:
                rr = slice(pc["r0"] + m * MT, pc["r0"] + (m + 1) * MT)
                s = np.maximum(row_s[rr] - pc["base"] - d * NT, 0)
                e = np.minimum(row_e[rr] - pc["base"] - d * NT, NT)
                ok = s < e
                if ok.any():
                    c0 = min(c0, int(s[ok].min()))
                    c1 = max(c1, int(e[ok].max()))
            if c1 <= c0:
                c0, c1 = 0, NT
            wm_.append((c0, c1))
        wins.append(wm_)
    # max classes per (m, diag-tile) block across cores (pois matmul k-dim)
    cp = 1
    for c in range(NCORES):
        pc = cores[c]
        r0, base = pc["r0"], pc["base"]
        for m in range(NMT):
            rr = slice(r0 + m * MT, r0 + (m + 1) * MT)
            ss = row_s[rr]
            for d in uni[m]:
                lo, hi = base + d * NT, base + (d + 1) * NT
                # classes whose column range intersects the bank
                cls = set()
                for g in range(rr.start, rr.stop):
                    if row_s[g] < hi and row_e[g] > lo:
                        cls.add(int(row_s[g]))
                cp = max(cp, len(cls))
    return dict(
        order=order, row_s=row_s, row_e=row_e, cores=cores, diag=uni,
        wins=wins, cp=cp,
    )


def _build_core_inputs(emb_norm: np.ndarray, plan, c: int):
    """emb_norm: label-sorted, unit-normalized embeddings (f32).
    Returns (xtn_rot [D,B], xbn [D,1024], pl [CP, nblk*MT],
    pr [CP, nblk*NT]) all bf16."""
    pc = plan["cores"][c]
    base, r0 = pc["base"], pc["r0"]
    rows_per_core = B // NCORES
    row_s, row_e = plan["row_s"], plan["row_e"]
    cp = plan["cp"]

    rot = np.r_[np.arange(base, B), np.arange(0, base)]
    xt_rot = np.ascontiguousarray(emb_norm[rot].T).astype(BF16)
    xb = np.ascontiguousarray(emb_norm[r0 : r0 + rows_per_core].T).astype(BF16)

    # poison matmul blocks: per (m, d in diag[m]):
    #   pl[k, i] = -2 if m-tile row i in class k else 0     [CP, MT]
    #   pr[k, j] = 1 if bank-d col j in class k else 0      [CP, NT]
    nblk = sum(len(d) for d in pc["diag"])
    pl = np.zeros((cp, nblk * MT), np.float32)
    pr = np.zeros((cp, nblk * NT), np.float32)
    bi = 0
    for m in range(NMT):
        for d in pc["diag"][m]:
            lo, hi = base + d * NT, base + (d + 1) * NT
            cls = {}
            for r in range(MT):
                g = r0 + m * MT + r
                s, e = int(row_s[g]), int(row_e[g])
                if s < hi and e > lo:
                    k = cls.setdefault(s, len(cls))
                    pl[k, bi * MT + r] = POIS
                    cs, ce = max(s - lo, 0), min(e - lo, NT)
                    pr[k, bi * NT + cs : bi * NT + ce] = 1.0
            assert len(cls) <= cp
            bi += 1
    pl = pl.astype(BF16)
    pr = pr.astype(BF16)

    return xt_rot, xb, pl, pr


# --------------------------------------------------------------------------
# device program
# --------------------------------------------------------------------------
def _raw_recip_bias(nc, out, in_, bias_ap):
    import concourse.mybir as mybir

    eng = nc.scalar
    ins = [
        eng.lower_ap(in_),
        eng.lower_ap(bias_ap),
        mybir.ImmediateValue(dtype=mybir.dt.float32, value=1.0),  # scale
        mybir.ImmediateValue(dtype=mybir.dt.float32, value=0.0),  # alpha
    ]
    return eng.add_instruction(
        mybir.InstActivation(
            name=f"I-{nc.next_id()}",
            func=mybir.ActivationFunctionType.Reciprocal,
            ins=ins,
            outs=[eng.lower_ap(out)],
        )
    )


def _build_bass(diag, wins, cp):
    import concourse.bacc as bacc
    import concourse.mybir as mybir
    from concourse.tile import TileContext

    f32 = mybir.dt.float32
    bf16 = mybir.dt.bfloat16
    i16 = mybir.dt.int16
    Alu = mybir.AluOpType
    Act = mybir.ActivationFunctionType
    NOH = N_NT + 2
    NBC = NMT * MT  # xb columns (1024)
    nblk = sum(len(d) for d in diag)

    nc = bacc.Bacc("TRN2", target_bir_lowering=False, debug=False, num_devices=NCORES)

    xt_d = nc.dram_tensor("xt", [D, B], bf16, kind="ExternalInput").ap()
    xb_d = nc.dram_tensor("xb", [D, NBC], bf16, kind="ExternalInput").ap()
    pl_d = nc.dram_tensor("pl", [cp, nblk * MT], bf16, kind="ExternalInput").ap()
    pr_d = nc.dram_tensor("pr", [cp, nblk * NT], bf16, kind="ExternalInput").ap()
    out_d = nc.dram_tensor("out", [MT, 2 * NMT], f32, kind="ExternalOutput").ap()
    outy_d = nc.dram_tensor("outy", [MT, NMT], bf16, kind="ExternalOutput").ap()

    # diag-block flat index per (m, d)
    blkof = {}
    bi = 0
    for m in range(NMT):
        for j, d in enumerate(diag[m]):
            blkof[(m, d)] = bi
            bi += 1

    with TileContext(nc) as tc:
        with (
            tc.tile_pool(name="big", bufs=1) as big,
            tc.tile_pool(name="upool", bufs=2) as upool,
            tc.tile_pool(name="ypool", bufs=2) as ypool,
            tc.tile_pool(name="scr", bufs=2) as scr,
            tc.tile_pool(name="sm", bufs=6) as smp,
            tc.tile_pool(name="psa", bufs=2, space="PSUM") as psa,
            tc.tile_pool(name="psp", bufs=2, space="PSUM") as psw,
        ):
            # ---------------- setup: load pre-normalized inputs -------------
            # xbn + first xtn chunks on the SP HWDGE queue; the back half of
            # xtn on the Activation HWDGE queue (two queues load in parallel)
            xbn = big.tile([D, NBC], bf16, tag="xbn")
            nc.sync.dma_start(xbn[:], xb_d)
            pl = big.tile([cp, nblk * MT], bf16, tag="pl")
            nc.sync.dma_start(pl[:], pl_d)
            pr = big.tile([cp, nblk * NT], bf16, tag="pr")
            nc.sync.dma_start(pr[:], pr_d)
            xtn = big.tile([D, B], bf16, tag="xtn")
            for j in range(4):
                sl = slice(j * (B // 8), (j + 1) * (B // 8))
                nc.sync.dma_start(xtn[:, sl], xt_d[:, sl])
            for j in range(4, 8):
                sl = slice(j * (B // 8), (j + 1) * (B // 8))
                nc.scalar.dma_start(xtn[:, sl], xt_d[:, sl])

            outb = big.tile([MT, 2 * NMT], f32, tag="outb")
            outy = big.tile([MT, NMT], bf16, tag="outy")
            ntpall = big.tile([MT, NMT], f32, tag="ntpall")

            # ---------------- main loop over M-tiles ----------------
            # per-bank slot in the u (Act) / y (Pool) buffers
            uslot = {b: i for i, b in enumerate(ACT_BANKS)}
            yslot = {b: i for i, b in enumerate(POOL_BANKS)}

            pending_tree = None  # (u, y, m) of previous M-tile

            def emit_trees(u, y, m):
                # u float min-tree: [MT, NA*NT] bf16 -> r1 -> outb[:, NMT+m]
                w = NA * NT  # 5120
                t1 = scr.tile([MT, w // 2], bf16, tag="ut1")
                nc.vector.tensor_tensor(t1[:], u[:, : w // 2], u[:, w // 2 :], Alu.min)
                t2 = scr.tile([MT, w // 4], bf16, tag="ut2")
                nc.vector.tensor_tensor(
                    t2[:], t1[:, : w // 4], t1[:, w // 4 :], Alu.min
                )
                t3 = scr.tile([MT, w // 8], bf16, tag="ut3")
                nc.vector.tensor_tensor(
                    t3[:], t2[:, : w // 8], t2[:, w // 8 :], Alu.min
                )
                t4 = scr.tile([MT, w // 16], bf16, tag="ut4")
                nc.vector.tensor_tensor(
                    t4[:], t3[:, : w // 16], t3[:, w // 16 :], Alu.min
                )
                t5 = scr.tile([MT, w // 32], bf16, tag="ut5")
                nc.vector.tensor_tensor(
                    t5[:], t4[:, : w // 32], t4[:, w // 32 :], Alu.min
                )
                nc.vector.tensor_reduce(
                    outb[:, NMT + m : NMT + m + 1], t5[:],
                    axis=mybir.AxisListType.X, op=Alu.min,
                )
                # y int16-bits min-tree: [MT, NP_*NT] bf16 -> outy[:, m]
                wy = NP_ * NT  # 3072
                yi = y[:].bitcast(i16)
                s1 = scr.tile([MT, wy // 2], i16, tag="yt1")
                nc.vector.tensor_tensor(
                    s1[:], yi[:, : wy // 2], yi[:, wy // 2 :], Alu.min
                )
                s2 = scr.tile([MT, wy // 4], i16, tag="yt2")
                nc.vector.tensor_tensor(
                    s2[:], s1[:, : wy // 4], s1[:, wy // 4 :], Alu.min
                )
                s3 = scr.tile([MT, wy // 8], i16, tag="yt3")
                nc.vector.tensor_tensor(
                    s3[:], s2[:, : wy // 8], s2[:, wy // 8 :], Alu.min
                )
                s4 = scr.tile([MT, wy // 16], i16, tag="yt4")
                nc.vector.tensor_tensor(
                    s4[:], s3[:, : wy // 16], s3[:, wy // 16 :], Alu.min
                )
                nc.vector.tensor_reduce(
                    outy[:, m : m + 1].bitcast(i16), s4[:],
                    axis=mybir.AxisListType.X, op=Alu.min,
                )

            def emit_tp(m):
                """Narrow-window diag matmuls (+pois) into one PSUM tile,
                then the t_p chain: Pool window mins -> DVE finalize."""
                dts = diag[m]
                lhsT = xbn[:, m * MT : (m + 1) * MT]
                tpp = outb[:, m : m + 1]
                ntp = ntpall[:, m : m + 1]
                minis = {}
                for j, d in enumerate(dts):
                    if j % GW == 0:
                        mini = psw.tile([MT, GW * NT], f32, tag="w")
                        minis[j // GW] = mini
                    jj = j % GW
                    c0, c1 = wins[m][j]
                    w = c1 - c0
                    i = blkof[(m, d)]
                    nc.tensor.matmul(
                        mini[:, jj * NT : jj * NT + w],
                        lhsT, xtn[:, d * NT + c0 : d * NT + c1],
                        start=True, stop=False,
                    )
                    nc.tensor.matmul(
                        mini[:, jj * NT : jj * NT + w],
                        pl[:, i * MT : (i + 1) * MT],
                        pr[:, i * NT + c0 : i * NT + c1],
                        start=False, stop=True,
                    )
                ndts = len(dts)
                posm = smp.tile([MT, max(ndts, 1)], f32, tag="posm")
                for j, d in enumerate(dts):
                    c0, c1 = wins[m][j]
                    jj = j % GW
                    nc.vector.tensor_reduce(
                        posm[:, j : j + 1],
                        minis[j // GW][:, jj * NT : jj * NT + (c1 - c0)],
                        axis=mybir.AxisListType.X, op=Alu.min,
                    )
                if ndts == 1:
                    minpos = posm[:, 0:1]
                else:
                    mp = smp.tile([MT, 1], f32, tag="minpos")
                    nc.vector.tensor_reduce(
                        mp[:], posm[:], axis=mybir.AxisListType.X, op=Alu.min
                    )
                    minpos = mp[:]
                # t_p = min(minpos - POIS, 1)
                nc.vector.tensor_scalar(
                    tpp, minpos, -POIS, 1.0, Alu.add, Alu.min
                )
                nc.vector.tensor_scalar_mul(ntp, tpp, -1.0)

            emit_tp(0)

            for m in range(NMT):
                dts = diag[m]
                lhsT = xbn[:, m * MT : (m + 1) * MT]
                tpp = outb[:, m : m + 1]       # +t_p ptr (f32)
                ntp = ntpall[:, m : m + 1]     # -t_p ptr (f32)
                u = upool.tile([MT, NA * NT], bf16, tag="u")
                y = ypool.tile([MT, NP_ * NT], bf16, tag="y")

                act_groups = sorted({b // GW for b in ACT_BANKS})
                pool_groups = sorted({b // GW for b in POOL_BANKS})
                for g in act_groups + pool_groups:
                    pool = psa if g in act_groups else psw
                    wg = pool.tile([MT, GW * NT], f32, tag="w")
                    for k in range(GW):
                        t = GW * g + k
                        if t in dts:
                            nc.tensor.matmul(
                                wg[:, k * NT : (k + 1) * NT],
                                lhsT, xtn[:, t * NT : (t + 1) * NT],
                                start=True, stop=False,
                            )
                            i = blkof[(m, t)]
                            nc.tensor.matmul(
                                wg[:, k * NT : (k + 1) * NT],
                                pl[:, i * MT : (i + 1) * MT],
                                pr[:, i * NT : (i + 1) * NT],
                                start=False, stop=True,
                            )
                        else:
                            nc.tensor.matmul(
                                wg[:, k * NT : (k + 1) * NT],
                                lhsT, xtn[:, t * NT : (t + 1) * NT],
                            )

                    # consumers (t_p for this m was precomputed)
                    bank0 = GW * g
                    ab = [b for b in range(bank0, bank0 + GW) if b in uslot]
                    if ab:
                        k0, k1 = ab[0] - bank0, ab[-1] - bank0 + 1
                        _raw_recip_bias(
                            nc,
                            u[:, uslot[ab[0]] * NT : uslot[ab[-1]] * NT + NT],
                            wg[:, k0 * NT : k1 * NT],
                            ntp,
                        )
                    pb = [b for b in range(bank0, bank0 + GW) if b in yslot]
                    if pb:
                        k0, k1 = pb[0] - bank0, pb[-1] - bank0 + 1
                        nc.gpsimd.tensor_scalar(
                            y[:, yslot[pb[0]] * NT : yslot[pb[-1]] * NT + NT],
                            wg[:, k0 * NT : k1 * NT],
                            tpp, tpp, Alu.min, Alu.subtract,
                        )

                # next M-tile's t_p runs on PE/Pool/DVE while this one's
                # Act/Pool streams are still draining
                if m + 1 < NMT:
                    emit_tp(m + 1)
                if pending_tree is not None:
                    emit_trees(*pending_tree)
                pending_tree = (u, y, m)

            emit_trees(*pending_tree)

            nc.sync.dma_start(out_d, outb[:])
            nc.sync.dma_start(outy_d, outy[:])

    nc.compile()
    return nc


# --------------------------------------------------------------------------
# entry point
# --------------------------------------------------------------------------
def _prepare(embeddings, labels):
    emb = np.asarray(embeddings, dtype=np.float32)
    lab = np.asarray(labels).astype(np.int64)
    plan = _plan(lab)
    emb_sorted = emb[plan["order"]]
    norm = np.linalg.norm(emb_sorted, axis=1, keepdims=True)
    emb_norm = emb_sorted / np.maximum(norm, 1e-12)
    cores = [_build_core_inputs(emb_norm, plan, c) for c in range(NCORES)]
    return emb, lab, plan, cores


def _host_reduce(emb, lab, plan, outs):
    """outs: per core {"out": [128, 16] f32, "outy": [128, 8] bf16}."""
    order = plan["order"]
    slab = lab[order]
    rows_per_core = B // NCORES

    t_p = np.zeros(B, np.float64)
    r1 = np.zeros(B, np.float64)
    yw = np.zeros(B, np.float64)
    for c in range(NCORES):
        o = np.asarray(outs[c]["out"], np.float64)
        oy = np.asarray(outs[c]["outy"]).astype(np.float64)
        for m in range(NMT):
            rr = slice(c * rows_per_core + m * MT, c * rows_per_core + (m + 1) * MT)
            t_p[rr] = o[:, m]
            r1[rr] = o[:, NMT + m]
            yw[rr] = oy[:, m]

    with np.errstate(divide="ignore", invalid="ignore"):
        q1 = t_p + 1.0 / r1
    q2 = t_p + yw
    c1 = (r1 < 0) & np.isfinite(q1)
    c2 = yw < 0
    q = np.where(
        c1 & c2, np.maximum(q1, q2), np.where(c1, q1, np.where(c2, q2, -np.inf))
    )
    d_ap = 1.0 - t_p
    d_semi = 1.0 - q
    lo = t_p - MARGIN

    # validity from class counts
    _, inv, counts = np.unique(slab, return_inverse=True, return_counts=True)
    cnt_row = counts[inv]
    valid = (cnt_row >= 2) & (cnt_row <= B - 1)

    EDGE = 1e-3
    semi_ok = (c1 | c2) & (q > lo + EDGE) & (q < t_p) & np.isfinite(q)
    redo = valid & ~semi_ok

    per_row = np.where(valid, np.maximum(d_ap - d_semi + MARGIN, 0.0), 0.0)

    if redo.any():
        e = emb / np.maximum(
            np.linalg.norm(emb, axis=1, keepdims=True), 1e-12
        )
        idx = order[np.flatnonzero(redo)]  # original row indices
        for g, i in zip(np.flatnonzero(redo), idx):
            dot = (e[i] @ e.T).astype(np.float32)
            dist = np.clip(1.0 - dot, 0.0, None)
            pos = (lab == lab[i])
            pos[i] = False
            neg = lab != lab[i]
            dap = dist[pos].max()
            semi = neg & (dist > dap) & (dist < dap + MARGIN)
            if semi.any():
                dan = dist[semi].min()
            else:
                dan = dist[neg].min()
            per_row[g] = max(dap - dan + MARGIN, 0.0)

    num_valid = max(int(valid.sum()), 1)
    loss = per_row[valid].sum() / num_valid
    return np.array(loss, dtype=np.float32)


def kernel_run(embeddings, labels, trace=False):
    import concourse.bass_utils as bass_utils

    emb, lab, plan, cores = _prepare(embeddings, labels)
    diag = plan["diag"]
    wins = plan["wins"]
    cp = plan["cp"]
    key = (
        tuple(tuple(d) for d in diag),
        tuple(tuple(w) for w in wins),
        cp,
    )
    if key not in _CACHE:
        _CACHE[key] = _build_bass(diag, wins, cp)
    nc = _CACHE[key]
    in_maps = [
        {"xt": np.ascontiguousarray(c[0]), "xb": np.ascontiguousarray(c[1]),
         "pl": np.ascontiguousarray(c[2]), "pr": np.ascontiguousarray(c[3])}
        for c in cores
    ]
    res = bass_utils.run_bass_kernel_spmd(
        nc, in_maps, core_ids=list(range(NCORES)), trace=trace
    )
    loss = _host_reduce(emb, lab, plan, res.results)
    return loss, res


def kernel(embeddings, labels):
    loss, _ = kernel_run(embeddings, labels)
    return loss


# revision 38
# speedup vs baseline: 1.7959x; 1.1056x over previous
"""Batch semi-hard triplet loss (cosine distance) on 8 Trainium2 NeuronCores.

Strategy (data-parallel over rows, per sharding hint):
  - Host: sort rows by label; core c takes sorted rows [1024c, 1024(c+1)) in
    8 exact 128-row M-tiles; columns rotated per core so its rows' class
    columns sit in the first PSUM group of each M-tile.
  - Device (per core, uniform SPMD program):
      * normalize embeddings (squares, one-hot column-sum matmuls, sqrt,
        reciprocal, one-hot broadcast matmuls, column scale);
      * per M-tile m: 16 matmuls (4-bank PSUM groups). Class-column poison
        (-2) is applied ON THE PE via small rank-per-class accumulate
        matmuls (lhsT = -2*row-indicators, rhs = col-indicators), so the
        diag group needs no mask adds. t_p (min positive-class dot) comes
        from a narrow poisoned window min on the first group.
        Then three engines split the threshold-max reduction
        q = max{dot < t_p}:
          - Act banks: u = 1/(dot - t_p) (Reciprocal w/ per-partition
            bias), bf16; float min-tree over u -> r1 (min u).
          - Pool banks: y = (dot min t_p) - t_p (one fused gpsimd
            tensor_scalar), bf16: candidates are negative, others +0;
            signed-int16 bit-pattern min over y picks the largest dot
            strictly below t_p (sign bit wraps the threshold).
          - DVE: runs both min-trees (u float-min, y int16-bits-min),
            one M-tile behind.
  - Host: q = max(t_p + 1/r1, t_p + y); per-row loss epilogue in f64;
    rows with no semi-hard candidate in the margin window (or near the
    branch boundary) are recomputed exactly in f32 numpy; mean over valid.
"""

import numpy as np
import ml_dtypes

B = 8192
D = 128
MARGIN = 0.2
NCORES = 8
NT = 512            # N-tile width (one PSUM bank of fp32)
N_NT = B // NT      # 16
MT = 128            # M-tile rows
NMT = B // NCORES // MT  # 8 m-tiles per core
GWA = 3             # banks per Act-family PSUM tile
POIS = -2.0         # class-column poison (exactly representable in bf16)

# bank families: Act does recip on banks 0..10; DVE shifts banks 11..15
# (GPSIMD cannot read PSUM on TRN2, so Pool only gets SBUF-side work)
ACT_BANKS = list(range(0, 11))
DVE_BANKS = [11, 12, 13, 14, 15]
NA = len(ACT_BANKS)
ND = len(DVE_BANKS)

BF16 = ml_dtypes.bfloat16

_CACHE = {}


# --------------------------------------------------------------------------
# host-side planning (pure layout, computed from labels)
# --------------------------------------------------------------------------
def _plan(labels: np.ndarray):
    order = np.argsort(labels, kind="stable")
    slab = labels[order]
    bounds = np.flatnonzero(np.r_[True, slab[1:] != slab[:-1], True])
    cls_start, cls_end = bounds[:-1], bounds[1:]
    row_s = np.empty(B, dtype=np.int64)
    row_e = np.empty(B, dtype=np.int64)
    for s, e in zip(cls_start, cls_end):
        row_s[s:e] = s
        row_e[s:e] = e

    rows_per_core = B // NCORES
    cores = []
    for c in range(NCORES):
        r0 = c * rows_per_core
        base = int(row_s[r0])  # start of first class -> no wraparound
        diag = []
        for m in range(NMT):
            rr = slice(r0 + m * MT, r0 + (m + 1) * MT)
            s = row_s[rr] - base
            e = row_e[rr] - base
            dts = sorted(set((s // NT).tolist()) | set(((e - 1) // NT).tolist()))
            diag.append(dts)
        cores.append(dict(r0=r0, base=base, diag=diag))
    # unify diag sets across cores so all 8 run one compiled program
    uni = [
        sorted(set().union(*[set(pc["diag"][m]) for pc in cores]))
        for m in range(NMT)
    ]
    for pc in cores:
        pc["diag"] = uni
    # per (m, diag tile): narrow column window [c0, c1) within the bank that
    # contains every class column of the tile's rows, across all cores
    wins = []
    for m in range(NMT):
        wm_ = []
        for d in uni[m]:
            c0, c1 = NT, 0
            for pc in cores:
                rr = slice(pc["r0"] + m * MT, pc["r0"] + (m + 1) * MT)
                s = np.maximum(row_s[rr] - pc["base"] - d * NT, 0)
                e = np.minimum(row_e[rr] - pc["base"] - d * NT, NT)
                ok = s < e
                if ok.any():
                    c0 = min(c0, int(s[ok].min()))
                    c1 = max(c1, int(e[ok].max()))
            if c1 <= c0:
                c0, c1 = 0, NT
            wm_.append((c0, c1))
        wins.append(wm_)
    # max classes per (m, diag-tile) block across cores (pois matmul k-dim)
    cp = 1
    for c in range(NCORES):
        pc = cores[c]
        r0, base = pc["r0"], pc["base"]
        for m in range(NMT):
            rr = slice(r0 + m * MT, r0 + (m + 1) * MT)
            ss = row_s[rr]
            for d in uni[m]:
                lo, hi = base + d * NT, base + (d + 1) * NT
                # classes whose column range intersects the bank
                cls = set()
                for g in range(rr.start, rr.stop):
                    if row_s[g] < hi and row_e[g] > lo:
                        cls.add(int(row_s[g]))
                cp = max(cp, len(cls))
    return dict(
        order=order, row_s=row_s, row_e=row_e, cores=cores, diag=uni,
        wins=wins, cp=cp,
    )


def _build_core_inputs(emb_norm: np.ndarray, plan, c: int):
    """emb_norm: label-sorted, unit-normalized embeddings (f32).
    Returns (xtn_rot [D,B], xbn [D,1024], pl [CP, nblk*MT],
    pr [CP, nblk*NT]) all bf16."""
    pc = plan["cores"][c]
    base, r0 = pc["base"], pc["r0"]
    rows_per_core = B // NCORES
    row_s, row_e = plan["row_s"], plan["row_e"]
    cp = plan["cp"]

    rot = np.r_[np.arange(base, B), np.arange(0, base)]
    xt_rot = np.ascontiguousarray(emb_norm[rot].T).astype(BF16)
    xb = np.ascontiguousarray(emb_norm[r0 : r0 + rows_per_core].T).astype(BF16)

    # poison matmul blocks: per (m, d in diag[m]):
    #   pl[k, i] = -2 if m-tile row i in class k else 0     [CP, MT]
    #   pr[k, j] = 1 if bank-d col j in class k else 0      [CP, NT]
    nblk = sum(len(d) for d in pc["diag"])
    pl = np.zeros((cp, nblk * MT), np.float32)
    pr = np.zeros((cp, nblk * NT), np.float32)
    bi = 0
    for m in range(NMT):
        for d in pc["diag"][m]:
            lo, hi = base + d * NT, base + (d + 1) * NT
            cls = {}
            for r in range(MT):
                g = r0 + m * MT + r
                s, e = int(row_s[g]), int(row_e[g])
                if s < hi and e > lo:
                    k = cls.setdefault(s, len(cls))
                    pl[k, bi * MT + r] = POIS
                    cs, ce = max(s - lo, 0), min(e - lo, NT)
                    pr[k, bi * NT + cs : bi * NT + ce] = 1.0
            assert len(cls) <= cp
            bi += 1
    pl = pl.astype(BF16)
    pr = pr.astype(BF16)

    return xt_rot, xb, pl, pr


# --------------------------------------------------------------------------
# device program
# --------------------------------------------------------------------------
def _raw_recip_bias(nc, out, in_, bias_ap):
    import concourse.mybir as mybir

    eng = nc.scalar
    ins = [
        eng.lower_ap(in_),
        eng.lower_ap(bias_ap),
        mybir.ImmediateValue(dtype=mybir.dt.float32, value=1.0),  # scale
        mybir.ImmediateValue(dtype=mybir.dt.float32, value=0.0),  # alpha
    ]
    return eng.add_instruction(
        mybir.InstActivation(
            name=f"I-{nc.next_id()}",
            func=mybir.ActivationFunctionType.Reciprocal,
            ins=ins,
            outs=[eng.lower_ap(out)],
        )
    )


def _build_bass(diag, wins, cp):
    import concourse.bacc as bacc
    import concourse.mybir as mybir
    from concourse.tile import TileContext

    f32 = mybir.dt.float32
    bf16 = mybir.dt.bfloat16
    i16 = mybir.dt.int16
    Alu = mybir.AluOpType
    Act = mybir.ActivationFunctionType
    NOH = N_NT + 2
    NBC = NMT * MT  # xb columns (1024)
    nblk = sum(len(d) for d in diag)

    nc = bacc.Bacc("TRN2", target_bir_lowering=False, debug=False, num_devices=NCORES)

    xt_d = nc.dram_tensor("xt", [D, B], bf16, kind="ExternalInput").ap()
    xb_d = nc.dram_tensor("xb", [D, NBC], bf16, kind="ExternalInput").ap()
    pl_d = nc.dram_tensor("pl", [cp, nblk * MT], bf16, kind="ExternalInput").ap()
    pr_d = nc.dram_tensor("pr", [cp, nblk * NT], bf16, kind="ExternalInput").ap()
    out_d = nc.dram_tensor("out", [MT, 2 * NMT], f32, kind="ExternalOutput").ap()
    outy_d = nc.dram_tensor("outy", [MT, NMT], bf16, kind="ExternalOutput").ap()

    # diag-block flat index per (m, d)
    blkof = {}
    bi = 0
    for m in range(NMT):
        for j, d in enumerate(diag[m]):
            blkof[(m, d)] = bi
            bi += 1

    with TileContext(nc) as tc:
        with (
            tc.tile_pool(name="big", bufs=1) as big,
            tc.tile_pool(name="upool", bufs=2) as upool,
            tc.tile_pool(name="ypool", bufs=2) as ypool,
            tc.tile_pool(name="sm", bufs=6) as smp,
            tc.tile_pool(name="psa", bufs=2, space="PSUM") as psa,
            tc.tile_pool(name="psp", bufs=2, space="PSUM") as psw,
        ):
            # ---------------- setup: load pre-normalized inputs -------------
            # xbn + first xtn chunks on the SP HWDGE queue; the back half of
            # xtn on the Activation HWDGE queue (two queues load in parallel)
            xbn = big.tile([D, NBC], bf16, tag="xbn")
            nc.sync.dma_start(xbn[:], xb_d)
            pl = big.tile([cp, nblk * MT], bf16, tag="pl")
            nc.sync.dma_start(pl[:], pl_d)
            pr = big.tile([cp, nblk * NT], bf16, tag="pr")
            nc.sync.dma_start(pr[:], pr_d)
            xtn = big.tile([D, B], bf16, tag="xtn")
            for j in range(4):
                sl = slice(j * (B // 8), (j + 1) * (B // 8))
                nc.sync.dma_start(xtn[:, sl], xt_d[:, sl])
            for j in range(4, 8):
                sl = slice(j * (B // 8), (j + 1) * (B // 8))
                nc.scalar.dma_start(xtn[:, sl], xt_d[:, sl])

            outb = big.tile([MT, 2 * NMT], f32, tag="outb")
            outy = big.tile([MT, NMT], bf16, tag="outy")
            ntpall = big.tile([MT, NMT], f32, tag="ntpall")

            # ---------------- main loop over M-tiles ----------------
            # Act-family groups of GWA banks; DVE-family single banks
            agroups = [
                ACT_BANKS[i : i + GWA] for i in range(0, NA, GWA)
            ]

            def emit_tp(m):
                """Narrow-window diag matmuls (+pois) into small PSUM tiles,
                then the t_p chain on DVE."""
                dts = diag[m]
                lhsT = xbn[:, m * MT : (m + 1) * MT]
                tpp = outb[:, m : m + 1]
                ntp = ntpall[:, m : m + 1]
                minis = []
                for j, d in enumerate(dts):
                    mini = psw.tile([MT, NT], f32, tag="w")
                    minis.append(mini)
                    c0, c1 = wins[m][j]
                    w = c1 - c0
                    i = blkof[(m, d)]
                    nc.tensor.matmul(
                        mini[:, 0:w],
                        lhsT, xtn[:, d * NT + c0 : d * NT + c1],
                        start=True, stop=False,
                    )
                    nc.tensor.matmul(
                        mini[:, 0:w],
                        pl[:, i * MT : (i + 1) * MT],
                        pr[:, i * NT + c0 : i * NT + c1],
                        start=False, stop=True,
                    )
                ndts = len(dts)
                posm = smp.tile([MT, max(ndts, 1)], f32, tag="posm")
                for j, d in enumerate(dts):
                    c0, c1 = wins[m][j]
                    nc.vector.tensor_reduce(
                        posm[:, j : j + 1],
                        minis[j][:, 0 : c1 - c0],
                        axis=mybir.AxisListType.X, op=Alu.min,
                    )
                if ndts == 1:
                    minpos = posm[:, 0:1]
                else:
                    mp = smp.tile([MT, 1], f32, tag="minpos")
                    nc.vector.tensor_reduce(
                        mp[:], posm[:], axis=mybir.AxisListType.X, op=Alu.min
                    )
                    minpos = mp[:]
                # t_p = min(minpos - POIS, 1)
                nc.vector.tensor_scalar(
                    tpp, minpos, -POIS, 1.0, Alu.add, Alu.min
                )
                nc.vector.tensor_scalar_mul(ntp, tpp, -1.0)

            def mm_bank(wg, ofs, t, m, lhsT):
                """main matmul for N-tile t into wg[:, ofs*NT:...], plus the
                class-poison accumulate matmul on diag banks."""
                if t in diag[m]:
                    nc.tensor.matmul(
                        wg[:, ofs * NT : (ofs + 1) * NT],
                        lhsT, xtn[:, t * NT : (t + 1) * NT],
                        start=True, stop=False,
                    )
                    i = blkof[(m, t)]
                    nc.tensor.matmul(
                        wg[:, ofs * NT : (ofs + 1) * NT],
                        pl[:, i * MT : (i + 1) * MT],
                        pr[:, i * NT : (i + 1) * NT],
                        start=False, stop=True,
                    )
                else:
                    nc.tensor.matmul(
                        wg[:, ofs * NT : (ofs + 1) * NT],
                        lhsT, xtn[:, t * NT : (t + 1) * NT],
                    )

            emit_tp(0)

            pending_min = None  # (u, y, m) of previous M-tile

            def emit_mins(u, y, m):
                # fused elementwise+accumulate-min: r1 = min(u) on DVE (4x),
                # ywin = int16-bits-min(y) on Pool (SBUF only)
                uj = upool.tile([MT, NA * NT], bf16, tag="uj")
                with nc.allow_low_precision(reason="u is bf16 by design"):
                    nc.vector.tensor_scalar(
                        uj[:], u[:], 1.0, None, Alu.mult, Alu.min,
                        accum_out=outb[:, NMT + m : NMT + m + 1],
                    )
                yj = ypool.tile([MT, ND * NT], i16, tag="yj")
                nc.gpsimd.tensor_scalar(
                    yj[:], y[:].bitcast(i16), 0, None, Alu.bitwise_or, Alu.min,
                    accum_out=outy[:, m : m + 1].bitcast(i16),
                )

            for m in range(NMT):
                lhsT = xbn[:, m * MT : (m + 1) * MT]
                tpp = outb[:, m : m + 1]       # +t_p ptr (f32)
                ntp = ntpall[:, m : m + 1]     # -t_p ptr (f32)
                u = upool.tile([MT, NA * NT], bf16, tag="u")
                y = ypool.tile([MT, ND * NT], bf16, tag="y")

                # Act-family groups first (Act is the pacing engine)
                for gi, banks in enumerate(agroups):
                    wg = psa.tile([MT, GWA * NT], f32, tag="w")
                    for k, t in enumerate(banks):
                        mm_bank(wg, k, t, m, lhsT)
                    u0 = ACT_BANKS.index(banks[0])
                    _raw_recip_bias(
                        nc,
                        u[:, u0 * NT : (u0 + len(banks)) * NT],
                        wg[:, 0 : len(banks) * NT],
                        ntp,
                    )
                # DVE-family banks: fused shift y = (dot min t_p) - t_p
                for di, t in enumerate(DVE_BANKS):
                    wg = psw.tile([MT, NT], f32, tag="w")
                    mm_bank(wg, 0, t, m, lhsT)
                    nc.vector.tensor_scalar(
                        y[:, di * NT : (di + 1) * NT], wg[:],
                        tpp, tpp, Alu.min, Alu.subtract,
                    )

                # next M-tile's t_p runs while this one's streams drain
                if m + 1 < NMT:
                    emit_tp(m + 1)
                if pending_min is not None:
                    emit_mins(*pending_min)
                pending_min = (u, y, m)

            emit_mins(*pending_min)

            nc.sync.dma_start(out_d, outb[:])
            nc.sync.dma_start(outy_d, outy[:])

    nc.compile()
    return nc


# --------------------------------------------------------------------------
# entry point
# --------------------------------------------------------------------------
def _prepare(embeddings, labels):
    emb = np.asarray(embeddings, dtype=np.float32)
    lab = np.asarray(labels).astype(np.int64)
    plan = _plan(lab)
    emb_sorted = emb[plan["order"]]
    norm = np.linalg.norm(emb_sorted, axis=1, keepdims=True)
    emb_norm = emb_sorted / np.maximum(norm, 1e-12)
    cores = [_build_core_inputs(emb_norm, plan, c) for c in range(NCORES)]
    return emb, lab, plan, cores


def _host_reduce(emb, lab, plan, outs):
    """outs: per core {"out": [128, 16] f32, "outy": [128, 8] bf16}."""
    order = plan["order"]
    slab = lab[order]
    rows_per_core = B // NCORES

    t_p = np.zeros(B, np.float64)
    r1 = np.zeros(B, np.float64)
    yw = np.zeros(B, np.float64)
    for c in range(NCORES):
        o = np.asarray(outs[c]["out"], np.float64)
        oy = np.asarray(outs[c]["outy"]).astype(np.float64)
        for m in range(NMT):
            rr = slice(c * rows_per_core + m * MT, c * rows_per_core + (m + 1) * MT)
            t_p[rr] = o[:, m]
            r1[rr] = o[:, NMT + m]
            yw[rr] = oy[:, m]

    with np.errstate(divide="ignore", invalid="ignore"):
        q1 = t_p + 1.0 / r1
    q2 = t_p + yw
    c1 = (r1 < 0) & np.isfinite(q1)
    c2 = yw < 0
    q = np.where(
        c1 & c2, np.maximum(q1, q2), np.where(c1, q1, np.where(c2, q2, -np.inf))
    )
    d_ap = 1.0 - t_p
    d_semi = 1.0 - q
    lo = t_p - MARGIN

    # validity from class counts
    _, inv, counts = np.unique(slab, return_inverse=True, return_counts=True)
    cnt_row = counts[inv]
    valid = (cnt_row >= 2) & (cnt_row <= B - 1)

    EDGE = 1e-3
    semi_ok = (c1 | c2) & (q > lo + EDGE) & (q < t_p) & np.isfinite(q)
    redo = valid & ~semi_ok

    per_row = np.where(valid, np.maximum(d_ap - d_semi + MARGIN, 0.0), 0.0)

    if redo.any():
        e = emb / np.maximum(
            np.linalg.norm(emb, axis=1, keepdims=True), 1e-12
        )
        idx = order[np.flatnonzero(redo)]  # original row indices
        for g, i in zip(np.flatnonzero(redo), idx):
            dot = (e[i] @ e.T).astype(np.float32)
            dist = np.clip(1.0 - dot, 0.0, None)
            pos = (lab == lab[i])
            pos[i] = False
            neg = lab != lab[i]
            dap = dist[pos].max()
            semi = neg & (dist > dap) & (dist < dap + MARGIN)
            if semi.any():
                dan = dist[semi].min()
            else:
                dan = dist[neg].min()
            per_row[g] = max(dap - dan + MARGIN, 0.0)

    num_valid = max(int(valid.sum()), 1)
    loss = per_row[valid].sum() / num_valid
    return np.array(loss, dtype=np.float32)


def kernel_run(embeddings, labels, trace=False):
    import concourse.bass_utils as bass_utils

    emb, lab, plan, cores = _prepare(embeddings, labels)
    diag = plan["diag"]
    wins = plan["wins"]
    cp = plan["cp"]
    key = (
        tuple(tuple(d) for d in diag),
        tuple(tuple(w) for w in wins),
        cp,
    )
    if key not in _CACHE:
        _CACHE[key] = _build_bass(diag, wins, cp)
    nc = _CACHE[key]
    in_maps = [
        {"xt": np.ascontiguousarray(c[0]), "xb": np.ascontiguousarray(c[1]),
         "pl": np.ascontiguousarray(c[2]), "pr": np.ascontiguousarray(c[3])}
        for c in cores
    ]
    res = bass_utils.run_bass_kernel_spmd(
        nc, in_maps, core_ids=list(range(NCORES)), trace=trace
    )
    loss = _host_reduce(emb, lab, plan, res.results)
    return loss, res


def kernel(embeddings, labels):
    loss, _ = kernel_run(embeddings, labels)
    return loss
